# revision 1
# baseline (speedup 1.0000x reference)
"""Trainium2 Bass kernel for nn_CombinedLoss (sinkhorn-KD + soft-CE + embed MSE).

Sharding (8 cores):
  - logits / batch: q-shard (each core owns a 128-wide q-slice of all 50 steps)
    -> per-core partial Gram matrices [128x128] over its D-shard of the
       flattened (t,q) feature axis, and partial CE gathers / `a` sums.
  - embed tensors: t-shard (7/7/6/..., zero-padded to 7).
  - one AllReduce of a packed [128,1800] partials buffer, then every core
    redundantly runs the (tiny) B x B sinkhorn iterations + CE + final combine.

The sinkhorn never materializes cost matrices: with C = 0.5|x|^2+0.5|y|^2-G and
the per-row term pulled out of the logsumexp, each softmin needs only
G/eps + h'_bcast, a segmented max / exp / sum, and rank-1 bookkeeping.
"""
import os
import numpy as np

B = 128
T = 50
Q = 1024
S = 49          # MAX_STEP - 1
H = 256
NCORES = 8
QS = Q // NCORES          # 128-wide q slice per core
TEMP = 0.5
GSCALE = 1.0 / (TEMP * TEMP)   # p-gram = GSCALE * logit-gram
RHO = 500.0 ** 2
EPS_FINAL = 0.005 ** 2
SUP_W, DIST_W, EMBED_W, LOSS_WEIGHT = 1.0, 0.01, 1.0, 1.0

# embed t-shard split (padded to 7 per core)
ESPLIT = [7, 7, 6, 6, 6, 6, 6, 6]
EOFF = [0, 7, 14, 20, 26, 32, 38, 44]
EPAD = 7

# arbuf layout (free axis, fp32 columns)
GALL0 = 0              # 3 pairs x [xx, xy, yx, yy] x 128
PCOFF = [1536, 1600, 1664]   # pc, pt, pe (64 cols each, 49 used)
AOFF = 1728            # sum(bc - bn) partial (64 cols, 49 used)
EMOFF = 1792           # embed partial column
ARF = 1800

CHUNKS = [(0, 10), (10, 10), (20, 10), (30, 10), (40, 10)]
GCH = [(0, 8), (8, 8), (16, 8), (24, 8), (32, 8), (40, 8), (48, 2)]


def _eps_schedule():
    eps_list = []
    e = 1.0
    while e > EPS_FINAL:
        eps_list.append(e)
        e = e * 0.25
    eps_list.append(EPS_FINAL)
    return eps_list


def build_bass():
    import concourse.bass as bass
    import concourse.bacc as bacc
    import concourse.tile as tile
    from concourse import mybir
    from concourse.masks import make_identity

    f32 = mybir.dt.float32
    f32r = mybir.dt.float32r
    bf16 = mybir.dt.bfloat16
    i32 = mybir.dt.int32
    Alu = mybir.AluOpType
    Act = mybir.ActivationFunctionType
    X = mybir.AxisListType.X

    nc = bacc.Bacc(
        "TRN2",
        target_bir_lowering=False,
        debug=False,
        num_devices=NCORES,
    )

    xs = [nc.declare_dram_parameter(n, [B, T, QS], f32, isOutput=False)
          for n in ("xc", "xt", "xe")]
    ys = [nc.declare_dram_parameter(n, [B, T, QS], f32, isOutput=False)
          for n in ("yc", "yt", "ye")]
    dbc = nc.declare_dram_parameter("dbc", [B, S, QS], f32, isOutput=False)
    dbn = nc.declare_dram_parameter("dbn", [B, S, QS], f32, isOutput=False)
    ehs = nc.declare_dram_parameter("ehs", [B, EPAD, H], f32, isOutput=False)
    eht = nc.declare_dram_parameter("eht", [B, EPAD, H], f32, isOutput=False)
    eds = nc.declare_dram_parameter("eds", [B, EPAD, H], f32, isOutput=False)
    edt = nc.declare_dram_parameter("edt", [B, EPAD, H], f32, isOutput=False)
    out_ext = nc.declare_dram_parameter("out", [1, 1], f32, isOutput=True)

    AR1F = 1024   # pairs 0/1 grams — reduced while pair 2 still computing
    AR2F = ARF - AR1F
    ar1_in = nc.dram_tensor("ar1_in", [B, AR1F], f32)
    ar1_out = nc.dram_tensor("ar1_out", [B, AR1F], f32, addr_space="Shared")
    ar2_in = nc.dram_tensor("ar2_in", [B, AR2F], f32)
    ar2_out = nc.dram_tensor("ar2_out", [B, AR2F], f32, addr_space="Shared")

    # constants baked into the NEFF
    msk_np = np.zeros((12, 1536), np.float32)
    for k in range(12):
        msk_np[k, 128 * k:128 * (k + 1)] = 1.0
    msk_dram = nc.inline_tensor(msk_np, "mskc")
    ckd = float(LOSS_WEIGHT * DIST_W * (RHO + EPS_FINAL / 2.0) / B)
    coeff_np = np.full((12, 1), -ckd, np.float32)
    coeff_np[0::4, 0] = ckd   # f_aa
    coeff_np[3::4, 0] = ckd   # g_bb
    coeff_dram = nc.inline_tensor(coeff_np, "coeffc")
    idx_np = np.broadcast_to(np.arange(64, dtype=np.float32), (B, 64)).copy()
    idx_dram = nc.inline_tensor(idx_np, "idxc")

    with tile.TileContext(nc) as tc:
        with tc.tile_pool(name="persist", bufs=1) as persist:
            ident = persist.tile([128, 128], f32)
            make_identity(nc, ident[:])
            arbuf = persist.tile([B, ARF], f32)
            nc.vector.memset(arbuf[:, 1536:ARF], 0.0)
            delta = persist.tile([B, S, QS], f32)

            # ---------------- phase A ----------------
            with (
                tc.tile_pool(name="loads", bufs=3) as loads,
                tc.tile_pool(name="bload", bufs=2) as bload,
                tc.tile_pool(name="b16", bufs=2) as b16,
                tc.tile_pool(name="rhsT", bufs=3) as rpool,
                tc.tile_pool(name="mul", bufs=2) as mpool,
                tc.tile_pool(name="epool", bufs=1) as epool,
                tc.tile_pool(name="gpsum", bufs=1, space="PSUM") as gpsum,
                tc.tile_pool(name="tpsum", bufs=3, space="PSUM") as tpsum,
            ):
                # delta + a partials from batch slices
                for (t0, w) in CHUNKS:
                    s1 = min(t0 + w, S)
                    ns = s1 - t0
                    if ns <= 0:
                        continue
                    bct = bload.tile([B, ns, QS], f32, tag="bc")
                    nc.sync.dma_start(out=bct[:], in_=dbc[:, t0:s1, :])
                    bnt = bload.tile([B, ns, QS], f32, tag="bn")
                    nc.sync.dma_start(out=bnt[:], in_=dbn[:, t0:s1, :])
                    nc.vector.tensor_add(delta[:, t0:s1, :], bct[:], bnt[:])
                    dif = bload.tile([B, ns, QS], f32, tag="dif")
                    nc.vector.tensor_sub(dif[:], bct[:], bnt[:])
                    nc.vector.reduce_sum(
                        out=arbuf[:, AOFF + t0:AOFF + s1], in_=dif[:], axis=X)

                # embed partials
                e1 = epool.tile([B, EPAD * H], f32, tag="ea")
                nc.sync.dma_start(out=e1[:], in_=ehs[:].rearrange("b t h -> b (t h)"))
                e2 = epool.tile([B, EPAD * H], f32, tag="eb")
                nc.sync.dma_start(out=e2[:], in_=eht[:].rearrange("b t h -> b (t h)"))
                ed = epool.tile([B, EPAD * H], f32, tag="ed")
                nc.vector.tensor_sub(ed[:], e1[:], e2[:])
                esq = epool.tile([B, EPAD * H], f32, tag="esq")
                ecols = persist.tile([B, 2], f32)
                nc.scalar.activation(esq[:], ed[:], Act.Square,
                                     accum_out=ecols[:, 0:1])
                e3 = epool.tile([B, EPAD * H], f32, tag="ea")
                nc.sync.dma_start(out=e3[:], in_=eds[:].rearrange("b t h -> b (t h)"))
                e4 = epool.tile([B, EPAD * H], f32, tag="eb")
                nc.sync.dma_start(out=e4[:], in_=edt[:].rearrange("b t h -> b (t h)"))
                ed2 = epool.tile([B, EPAD * H], f32, tag="ed")
                nc.vector.tensor_sub(ed2[:], e3[:], e4[:])
                esq2 = epool.tile([B, EPAD * H], f32, tag="esq")
                nc.scalar.activation(esq2[:], ed2[:], Act.Square,
                                     accum_out=ecols[:, 1:2])
                nc.vector.tensor_add(arbuf[:, EMOFF:EMOFF + 1],
                                     ecols[:, 0:1], ecols[:, 1:2])

                # grams + CE gathers (bf16 transpose/matmul pipeline)
                ident16 = persist.tile([128, 128], bf16)
                nc.vector.tensor_copy(ident16[:], ident[:])
                for p in range(3):
                    gpa = gpsum.tile([128, 256], f32, tag="ga")
                    gpb = gpsum.tile([128, 256], f32, tag="gb")
                    for (t0, w) in GCH:
                        xt_ = loads.tile([B, w, QS], f32, tag="xc")
                        nc.sync.dma_start(out=xt_[:], in_=xs[p][:, t0:t0 + w, :])
                        yt_ = loads.tile([B, w, QS], f32, tag="yc")
                        nc.sync.dma_start(out=yt_[:], in_=ys[p][:, t0:t0 + w, :])
                        xb = b16.tile([B, w, QS], bf16, tag="xb")
                        nc.scalar.copy(xb[:], xt_[:])
                        yb = b16.tile([B, w, QS], bf16, tag="yb")
                        nc.scalar.copy(yb[:], yt_[:])
                        for g0 in range(0, w, 4):
                            gw = min(4, w - g0)
                            bx = tpsum.tile([128, 512], bf16, tag="bx")
                            by = tpsum.tile([128, 512], bf16, tag="by")
                            for j in range(gw):
                                nc.tensor.transpose(bx[:, 128 * j:128 * (j + 1)],
                                                    xb[:, g0 + j, :], ident16[:])
                                nc.tensor.transpose(by[:, 128 * j:128 * (j + 1)],
                                                    yb[:, g0 + j, :], ident16[:])
                            rbig = rpool.tile([128, 2, 512], bf16, tag="r")
                            nc.scalar.copy(rbig[:, 0, 0:128 * gw],
                                           bx[:, 0:128 * gw])
                            nc.scalar.copy(rbig[:, 1, 0:128 * gw],
                                           by[:, 0:128 * gw])
                            for j in range(gw):
                                kk = t0 + g0 + j
                                rhs_j = rbig[:, :, 128 * j:128 * (j + 1)]
                                nc.tensor.matmul(gpa[:], rbig[:, 0, 128 * j:128 * (j + 1)],
                                                 rhs_j, start=(kk == 0),
                                                 stop=(kk == T - 1))
                                nc.tensor.matmul(gpb[:], rbig[:, 1, 128 * j:128 * (j + 1)],
                                                 rhs_j, start=(kk == 0),
                                                 stop=(kk == T - 1))
                        s1 = min(t0 + w, S)
                        if t0 < S:
                            ns = s1 - t0
                            ms = mpool.tile([B, w, QS], f32, tag="m")
                            nc.vector.tensor_mul(ms[:, 0:ns, :], xt_[:, 0:ns, :],
                                                 delta[:, t0:s1, :])
                            nc.vector.reduce_sum(
                                out=arbuf[:, PCOFF[p] + t0:PCOFF[p] + s1],
                                in_=ms[:, 0:ns, :], axis=X)
                    nc.scalar.copy(arbuf[:, 512 * p:512 * p + 256], gpa[:])
                    nc.scalar.copy(arbuf[:, 512 * p + 256:512 * (p + 1)], gpb[:])

            # ---------------- AllReduce (split: AR1 overlaps pair 2) -----
            nc.sync.dma_start(out=ar1_in[:, :], in_=arbuf[:, 0:1024])
            nc.gpsimd.collective_compute(
                "AllReduce",
                mybir.AluOpType.add,
                replica_groups=[list(range(NCORES))],
                ins=[ar1_in[:, :]],
                outs=[ar1_out[:, :]],
            )
            nc.sync.dma_start(out=ar2_in[:, :], in_=arbuf[:, 1024:ARF])
            nc.gpsimd.collective_compute(
                "AllReduce",
                mybir.AluOpType.add,
                replica_groups=[list(range(NCORES))],
                ins=[ar2_in[:, :]],
                outs=[ar2_out[:, :]],
            )
            post = persist.tile([B, ARF], f32)
            nc.sync.dma_start(out=post[:, 0:1024], in_=ar1_out[:, :])
            nc.sync.dma_start(out=post[:, 1024:ARF], in_=ar2_out[:, :])

            # ---------------- phase B ----------------
            with (
                tc.tile_pool(name="pbig", bufs=2) as pbig,
                tc.tile_pool(name="psmall", bufs=2) as psmall,
                tc.tile_pool(name="pconst", bufs=1) as pconst,
                tc.tile_pool(name="hps", bufs=3, space="PSUM") as hpsum,
                tc.tile_pool(name="fps", bufs=1, space="PSUM") as fpsum,
                tc.tile_pool(name="sps", bufs=1, space="PSUM") as spsum,
            ):
                # diag extraction: dvec cols [dxx0,dyy0,dxx1,dyy1,dxx2,dyy2]
                dvec = pconst.tile([B, 6], f32)
                for p in range(3):
                    for bi, col in ((0, 2 * p), (3, 2 * p + 1)):
                        blk = post[:, 512 * p + 128 * bi:512 * p + 128 * (bi + 1)]
                        dsc = psmall.tile([B, 128], f32, tag="dsc")
                        nc.vector.tensor_mul(dsc[:], blk, ident[:])
                        nc.vector.reduce_sum(out=dvec[:, col:col + 1], in_=dsc[:],
                                             axis=X)
                # D2 (row diag, blocks [xx,xy,yx,yy]) and DH (h-side diag, *-2)
                D2 = pconst.tile([B, 12], f32)
                DH = pconst.tile([B, 12], f32)
                for p in range(3):
                    dxx = dvec[:, 2 * p:2 * p + 1]
                    dyy = dvec[:, 2 * p + 1:2 * p + 2]
                    for col, src in ((0, dxx), (1, dxx), (2, dyy), (3, dyy)):
                        nc.vector.tensor_scalar_mul(D2[:, 4 * p + col:4 * p + col + 1],
                                                    src, 2.0)
                    for col, src in ((0, dxx), (1, dyy), (2, dxx), (3, dyy)):
                        nc.vector.tensor_scalar_mul(DH[:, 4 * p + col:4 * p + col + 1],
                                                    src, -2.0)

                mskt = pconst.tile([12, 1536], f32)
                nc.sync.dma_start(out=mskt[:], in_=msk_dram[:, :])
                ones12f = pconst.tile([12, 128], f32)
                nc.vector.memset(ones12f[:], 1.0)
                ones12 = pconst.tile([12, 128], f32r)
                nc.vector.tensor_copy(ones12[:], ones12f[:])
                ones_col = pconst.tile([B, 1], f32)
                nc.vector.memset(ones_col[:], 1.0)
                F = pconst.tile([B, 12], f32)
                nc.vector.memset(F[:], 0.0)

                blog = float(-np.log(float(B)))
                idr = pconst.tile([128, 128], f32r)
                nc.vector.tensor_copy(idr[:], ident[:])
                Gsb = pconst.tile([B, 1536], f32r)
                nc.vector.tensor_copy(Gsb[:], post[:, 0:1536])

                for eps in _eps_schedule():
                    damp = 1.0 / (1.0 + eps / RHO)
                    c = GSCALE / eps
                    # HT'' = ((F + DH)^T) * 0.25 + blog*eps/GSCALE   [12,128]
                    fsum = psmall.tile([B, 12], f32, tag="fsum")
                    nc.vector.tensor_add(fsum[:], F[:], DH[:])
                    ftp = fpsum.tile([12, 128], f32, tag="ft")
                    nc.tensor.transpose(ftp[:], fsum[:], ident[:])
                    HT = psmall.tile([12, 128], f32, tag="ht")
                    nc.vector.tensor_scalar(HT[:], ftp[:], 0.25,
                                            blog * eps / GSCALE,
                                            Alu.mult, Alu.add)
                    # T1' = G + H''_bcast in PSUM (3 banks x [128,512])
                    hb = []
                    HTQ = HT[:].unsqueeze(1).broadcast_to((12, 4, 128))
                    for p in range(3):
                        hbt = hpsum.tile([128, 512], f32, tag="hb")
                        hb.append(hbt)
                        rhm = psmall.tile([12, 4, 128], f32r, tag="rhm")
                        nc.vector.tensor_tensor(
                            rhm[:], HTQ,
                            mskt[:, 512 * p:512 * (p + 1)].rearrange(
                                "k (a j) -> k a j", j=128),
                            Alu.mult)
                        nc.tensor.matmul(hbt[:], ones12[:],
                                         rhm[:].rearrange("k a j -> k (a j)"),
                                         start=True, stop=False)
                        nc.tensor.matmul(hbt[:], idr[:],
                                         Gsb[:, 512 * p:512 * (p + 1)],
                                         start=False, stop=True)
                    mv = psmall.tile([B, 12], f32, tag="mv")
                    scr = pbig.tile([B, 12, 128], f32, tag="scr")
                    for p in range(3):
                        hb3 = hb[p][:].rearrange("b (s q) -> b s q", q=128)
                        nc.vector.reduce_max(out=mv[:, 4 * p:4 * p + 4], in_=hb3,
                                             axis=X)
                        mb = mv[:, 4 * p:4 * p + 4].unsqueeze(2).broadcast_to(
                            (B, 4, 128))
                        nc.vector.tensor_tensor(scr[:, 4 * p:4 * p + 4, :], hb3, mb,
                                                Alu.subtract)
                    scre = pbig.tile([B, 12, 128], f32, tag="scre")
                    sv = psmall.tile([B, 12], f32, tag="sv")
                    for p in range(3):
                        nc.scalar.activation(scre[:, 4 * p:4 * p + 4, :],
                                             scr[:, 4 * p:4 * p + 4, :],
                                             Act.Exp, scale=float(c))
                        nc.vector.reduce_sum(out=sv[:, 4 * p:4 * p + 4],
                                             in_=scre[:, 4 * p:4 * p + 4, :],
                                             axis=X)
                    # ln(sv) on DVE: exponent/mantissa split + deg-5 poly
                    LN2 = 0.6931471805599453
                    PA = (0.99988786, -0.49636758, 0.30467027, -0.15602615,
                          0.04106372)
                    svi = sv[:].bitcast(i32)
                    sh = psmall.tile([B, 12], i32, tag="lsh")
                    nc.vector.tensor_scalar(sh[:], svi, 23, None,
                                            Alu.logical_shift_right)
                    ef = psmall.tile([B, 12], f32, tag="lef")
                    nc.vector.tensor_copy(ef[:], sh[:])
                    mi = psmall.tile([B, 12], i32, tag="lmi")
                    nc.vector.tensor_scalar(mi[:], svi, 0x007FFFFF, 0x3F800000,
                                            Alu.bitwise_and, Alu.bitwise_or)
                    tt_ = psmall.tile([B, 12], f32, tag="ltt")
                    nc.vector.tensor_scalar(tt_[:], mi[:].bitcast(f32), 1.0, None,
                                            Alu.subtract)
                    hp = psmall.tile([B, 12], f32, tag="lhp")
                    nc.vector.tensor_scalar(hp[:], tt_[:], PA[4], PA[3],
                                            Alu.mult, Alu.add)
                    for ak in (PA[2], PA[1], PA[0]):
                        hm = psmall.tile([B, 12], f32, tag="lhm")
                        nc.vector.tensor_tensor(hm[:], hp[:], tt_[:], Alu.mult)
                        hp = psmall.tile([B, 12], f32, tag="lhp")
                        nc.vector.tensor_scalar(hp[:], hm[:], ak, None, Alu.add)
                    pv = psmall.tile([B, 12], f32, tag="lpv")
                    nc.vector.tensor_tensor(pv[:], hp[:], tt_[:], Alu.mult)
                    e2f = psmall.tile([B, 12], f32, tag="le2")
                    nc.vector.tensor_scalar(e2f[:], ef[:], LN2, -127.0 * LN2,
                                            Alu.mult, Alu.add)
                    lg = psmall.tile([B, 12], f32, tag="lg")
                    nc.vector.tensor_tensor(lg[:], e2f[:], pv[:], Alu.add)
                    # cand = damp * (D2 - 4m - eps*log s)
                    m4 = psmall.tile([B, 12], f32, tag="m4")
                    nc.vector.tensor_scalar_mul(m4[:], mv[:], 4.0)
                    u = psmall.tile([B, 12], f32, tag="u")
                    nc.vector.scalar_tensor_tensor(u[:], lg[:], float(eps), m4[:],
                                                   Alu.mult, Alu.add)
                    dmu = psmall.tile([B, 12], f32, tag="dmu")
                    nc.vector.tensor_tensor(dmu[:], D2[:], u[:], Alu.subtract)
                    cand = psmall.tile([B, 12], f32, tag="cand")
                    nc.vector.tensor_scalar_mul(cand[:], dmu[:], float(damp))
                    # state update; cols per pair [f_aa, g_ab, f_ab, g_bb]
                    F4 = F[:].rearrange("b (pr c) -> b pr c", c=4)
                    C4 = cand[:].rearrange("b (pr c) -> b pr c", c=4)
                    for col in (0, 3):     # averaging cols (f_aa, g_bb)
                        t_ = psmall.tile([B, 3], f32, tag="t_")
                        nc.vector.tensor_add(t_[:], F4[:, :, col], C4[:, :, col])
                        nc.vector.tensor_scalar_mul(F4[:, :, col], t_[:], 0.5)
                    nc.vector.tensor_copy(F4[:, :, 2], C4[:, :, 1])  # f_ab <- xy
                    nc.vector.tensor_copy(F4[:, :, 1], C4[:, :, 2])  # g_ab <- yx

                # ---- loss_kd ----
                E2 = psmall.tile([B, 12], f32, tag="e2")
                nc.scalar.activation(E2[:], F[:], Act.Exp, scale=float(-1.0 / RHO))
                cs_ps = spsum.tile([12, 1], f32, tag="cs")
                nc.tensor.matmul(cs_ps[:], E2[:], ones_col[:], start=True, stop=True)
                cs = psmall.tile([12, 1], f32, tag="css")
                nc.vector.tensor_copy(cs[:], cs_ps[:])
                coeff = pconst.tile([12, 1], f32)
                nc.sync.dma_start(out=coeff[:], in_=coeff_dram[:, :])

                # ---- CE ----
                idxf = pconst.tile([B, 64], f32)
                nc.sync.dma_start(out=idxf[:], in_=idx_dram[:, :])
                pcb = post[:, PCOFF[0]:PCOFF[0] + 64]
                pos = psmall.tile([B, 64], f32, tag="pos")
                nc.vector.tensor_scalar(pos[:], pcb, 0.0, None, Alu.is_gt)
                ip1 = psmall.tile([B, 64], f32, tag="ip1")
                nc.vector.scalar_tensor_tensor(ip1[:], idxf[:], 1.0, pos[:],
                                               Alu.add, Alu.mult)
                Lp = psmall.tile([B, 1], f32, tag="Lp")
                nc.vector.reduce_max(out=Lp[:], in_=ip1[:], axis=X)
                eq0 = psmall.tile([B, 1], f32, tag="eq0")
                nc.vector.tensor_scalar(eq0[:], Lp[:], 0.0, None, Alu.is_equal)
                Lv = psmall.tile([B, 1], f32, tag="Lv")
                nc.vector.scalar_tensor_tensor(Lv[:], eq0[:], float(S), Lp[:],
                                               Alu.mult, Alu.add)
                dl = psmall.tile([B, 64], f32, tag="dl")
                nc.vector.tensor_scalar(dl[:], idxf[:], Lv[:, 0:1], None,
                                        Alu.subtract)
                mask = psmall.tile([B, 64], f32, tag="mask")
                nc.vector.tensor_scalar(mask[:], dl[:], 0.0, None, Alu.is_lt)
                negf = psmall.tile([B, 64], f32, tag="negf")
                nc.vector.tensor_scalar(negf[:], mask[:], 1.0, 1e9,
                                        Alu.subtract, Alu.mult)
                # a = floor((asum+1)/2).  asum is integer-valued, so
                # t = asum*0.5 + 1024.25 has frac in {.25,.75}; round-to-
                # nearest-even(t) - .25-shift == floor, computed exactly via
                # the 1.5*2^23 magic add/sub (values stay < 2^22).
                MAGIC = 12582912.0
                tv = psmall.tile([B, 64], f32, tag="tv")
                nc.vector.tensor_scalar(tv[:], post[:, AOFF:AOFF + 64], 0.5,
                                        1024.25, Alu.mult, Alu.add)
                tm = psmall.tile([B, 64], f32, tag="tm")
                nc.vector.tensor_scalar(tm[:], tv[:], MAGIC, MAGIC,
                                        Alu.add, Alu.subtract)
                av = psmall.tile([B, 64], f32, tag="av")
                nc.vector.tensor_scalar(av[:], tm[:], 1024.0, None, Alu.subtract)
                amask = psmall.tile([B, 64], f32, tag="amask")
                nc.vector.tensor_tensor(amask[:], av[:], mask[:], Alu.mult)
                # m_ce over [B, 3, 64]
                pc3 = post[:, PCOFF[0]:PCOFF[0] + 192].rearrange(
                    "b (s q) -> b s q", q=64)
                mce = pbig.tile([B, 3, 64], f32, tag="mce")
                mask3 = mask[:].unsqueeze(1).broadcast_to((B, 3, 64))
                negf3 = negf[:].unsqueeze(1).broadcast_to((B, 3, 64))
                amask3 = amask[:].unsqueeze(1).broadcast_to((B, 3, 64))
                t2_ = pbig.tile([B, 3, 64], f32, tag="tt")
                nc.vector.scalar_tensor_tensor(t2_[:], pc3, 2.0, mask3, Alu.mult,
                                               Alu.mult)
                nc.vector.tensor_tensor(mce[:], t2_[:], negf3, Alu.add)
                mx3 = psmall.tile([B, 3], f32, tag="mx3")
                nc.vector.reduce_max(out=mx3[:], in_=mce[:], axis=X)
                mb3 = mx3[:].unsqueeze(2).broadcast_to((B, 3, 64))
                dd = pbig.tile([B, 3, 64], f32, tag="dd")
                nc.vector.tensor_tensor(dd[:], mce[:], mb3, Alu.subtract)
                ee = pbig.tile([B, 3, 64], f32, tag="ee")
                nc.scalar.activation(ee[:], dd[:], Act.Exp)
                ss3 = psmall.tile([B, 3], f32, tag="ss3")
                nc.vector.reduce_sum(out=ss3[:], in_=ee[:], axis=X)
                lg3 = psmall.tile([B, 3], f32, tag="lg3")
                nc.scalar.activation(lg3[:], ss3[:], Act.Ln)
                lse3 = psmall.tile([B, 3], f32, tag="lse3")
                nc.vector.tensor_add(lse3[:], mx3[:], lg3[:])
                lb3 = lse3[:].unsqueeze(2).broadcast_to((B, 3, 64))
                d1 = pbig.tile([B, 3, 64], f32, tag="dd")
                nc.vector.tensor_tensor(d1[:], mce[:], lb3, Alu.subtract)
                d2_ = pbig.tile([B, 3, 64], f32, tag="tt")
                nc.vector.tensor_tensor(d2_[:], d1[:], amask3, Alu.mult)
                rowsum = psmall.tile([B, 1], f32, tag="rs")
                nc.vector.reduce_sum(out=rowsum[:],
                                     in_=d2_[:].rearrange("b s q -> b (s q)"),
                                     axis=X)

                # ---- final combine into one PSUM scalar ----
                csup = pconst.tile([B, 1], f32)
                nc.vector.memset(csup[:], float(-LOSS_WEIGHT * SUP_W))
                cemb = pconst.tile([B, 1], f32)
                nc.vector.memset(cemb[:], float(LOSS_WEIGHT * EMBED_W * 0.5))
                tot_ps = spsum.tile([1, 1], f32, tag="tot")
                nc.tensor.matmul(tot_ps[:], rowsum[:], csup[:], start=True,
                                 stop=False)
                nc.tensor.matmul(tot_ps[:], post[:, EMOFF:EMOFF + 1], cemb[:],
                                 start=False, stop=False)
                nc.tensor.matmul(tot_ps[:], cs[:], coeff[:], start=False, stop=True)
                outt = psmall.tile([1, 1], f32, tag="outt")
                nc.vector.tensor_copy(outt[:], tot_ps[:])
                nc.sync.dma_start(out=out_ext[:, :], in_=outt[:])

    nc.compile()
    return nc


_NC = None
LAST_RESULTS = None


def _shard_inputs(logit_c, logit_t, logit_ensemble, logit_teacher_c,
                  logit_teacher_t, logit_teacher_ensemble, out_h_student,
                  out_h_teacher, out_d_student, out_d_teacher, batch):
    asf = lambda a: np.ascontiguousarray(a, dtype=np.float32)
    students = [logit_c, logit_t, logit_ensemble]
    teachers = [logit_teacher_c, logit_teacher_t, logit_teacher_ensemble]
    embeds = dict(ehs=out_h_student, eht=out_h_teacher,
                  eds=out_d_student, edt=out_d_teacher)
    in_maps = []
    for c in range(NCORES):
        q0 = QS * c
        m = {}
        for nm, arr in zip(("xc", "xt", "xe"), students):
            m[nm] = asf(arr[:, :, q0:q0 + QS])
        for nm, arr in zip(("yc", "yt", "ye"), teachers):
            m[nm] = asf(arr[:, :, q0:q0 + QS])
        m["dbc"] = asf(batch[:, 1:1 + S, q0:q0 + QS])
        m["dbn"] = asf(batch[:, 1:1 + S, Q + q0:Q + q0 + QS])
        t0, w = EOFF[c], ESPLIT[c]
        for nm, arr in embeds.items():
            sl = np.zeros((B, EPAD, H), np.float32)
            sl[:, :w, :] = np.asarray(arr[:, t0:t0 + w, :], dtype=np.float32)
            m[nm] = sl
        in_maps.append(m)
    return in_maps


def kernel(**inputs):
    global _NC, LAST_RESULTS
    from concourse.bass_utils import run_bass_kernel_spmd
    if _NC is None:
        _NC = build_bass()
    in_maps = _shard_inputs(**inputs)
    trace = bool(int(os.environ.get("KERNEL_TRACE", "0")))
    res = run_bass_kernel_spmd(_NC, in_maps, list(range(NCORES)), trace=trace)
    LAST_RESULTS = res
    return np.asarray(res.results[0]["out"], dtype=np.float32).reshape(1)



# revision 8
# speedup vs baseline: 1.4240x; 1.4240x over previous
"""Trainium2 Bass kernel for nn_CombinedLoss (sinkhorn-KD + soft-CE + embed MSE).

v2 architecture (8 cores):
  - All inputs shipped bf16. Logits in q-major [B, QS, T] per-core layout so a
    single XBAR DMA-transpose per tensor yields [q, t, b] gram operands
    (no PE transposes, no PSUM evacuation of transposed data).
  - Grams per pair: [G_xx | G_xy] and G_yy only; G_yx derived on the owner
    core as G_xy^T after reduction.
  - Two bf16 AllReduces in [mat*128+b, j] block layout: C1 (pairs 0/1 grams +
    CE/a/embed/diag cols) fires after pair 1 and hides under pair-2 compute;
    C2 (pair-2 grams + pc2/diag2) is the only exposed collective.
  - Phase B (9 sinkhorn iterations) sharded: each core processes only 2 of
    the 12 B x B matrices, selected via per-core indirect-DMA row gathers and
    data-driven update rules (uniform SPMD program).
  - Final loss_kd partials stay per-core; the host sums the 8 scalar outputs
    (the unshard step - all loss reductions are sums over shards).
"""
import os
import numpy as np

B = 128
T = 50
Q = 1024
S = 49          # MAX_STEP - 1
H = 256
NCORES = 8
QS = Q // NCORES
TEMP = 0.5
GSCALE = 1.0 / (TEMP * TEMP)   # p-gram = GSCALE * logit-gram = 4
RHO = 500.0 ** 2
EPS_FINAL = 0.005 ** 2
SUP_W, DIST_W, EMBED_W, LOSS_WEIGHT = 1.0, 0.01, 1.0, 1.0
CKD = float(LOSS_WEIGHT * DIST_W * (RHO + EPS_FINAL / 2.0) / B)

# embed t-shard split (padded to 7 per core)
ESPLIT = [7, 7, 6, 6, 6, 6, 6, 6]
EOFF = [0, 7, 14, 20, 26, 32, 38, 44]
EPAD = 7

# c1 AllReduce buffer: [1024, 128] bf16 rows; mat k occupies rows 128k..128k+127
# mats: xy0=0, xy1=1, xx0=2, xx1=3, yy0=4, yy1=5; CE-extra blocks rows 768-1023
# CE-extra cols (within the logical [B, 256] block):
#   pc0 at 0-63, pc1 64-127, a 128-191, embed 192, diag01 193-196
C1ROWS = 1024
# c2: xy2=0, xx2=1, yy2=2; CE2-extra rows 384-511: pc2 0-63, diag2 64-65
C2ROWS = 512

C1MAT = {('xy', 0): 0, ('xy', 1): 1, ('xx', 0): 2, ('xx', 1): 3,
         ('yy', 0): 4, ('yy', 1): 5}
C2MAT = {('xy', 2): 0, ('xx', 2): 1, ('yy', 2): 2}

# per-core slot assignment: (slot0 src, slot1 src); None = derived/dead
# a0=1: slot0 <- c1 read; a0=0: slot0 <- transpose(c2 read). etc.
SLOTS = {
    0: dict(i0=('xy', 0), i1=None, a0=1, a1=0, pf=1,
            kc=[-CKD, -CKD], rs=[('x', 0), ('y', 0)], cs=[('y', 0), ('x', 0)]),
    1: dict(i0=('xy', 1), i1=None, a0=1, a1=0, pf=1,
            kc=[-CKD, -CKD], rs=[('x', 1), ('y', 1)], cs=[('y', 1), ('x', 1)]),
    2: dict(i0=None, i1=('xy', 2), a0=0, a1=1, pf=1,
            kc=[-CKD, -CKD], rs=[('y', 2), ('x', 2)], cs=[('x', 2), ('y', 2)]),
    3: dict(i0=('xx', 0), i1=('xy', 2), a0=1, a1=1, pf=0,
            kc=[CKD, 0.0], rs=[('x', 0), ('x', 2)], cs=[('x', 0), ('y', 2)]),
    4: dict(i0=('xx', 1), i1=('xy', 2), a0=1, a1=1, pf=0,
            kc=[CKD, 0.0], rs=[('x', 1), ('x', 2)], cs=[('x', 1), ('y', 2)]),
    5: dict(i0=('xy', 0), i1=('xx', 2), a0=1, a1=1, pf=0,
            kc=[0.0, CKD], rs=[('x', 0), ('x', 2)], cs=[('y', 0), ('x', 2)]),
    6: dict(i0=('yy', 0), i1=('yy', 2), a0=1, a1=1, pf=0,
            kc=[CKD, CKD], rs=[('y', 0), ('y', 2)], cs=[('y', 0), ('y', 2)]),
    7: dict(i0=('yy', 1), i1=('xy', 2), a0=1, a1=1, pf=0,
            kc=[CKD, 0.0], rs=[('y', 1), ('y', 2)], cs=[('y', 1), ('y', 2)]),
}
# diag6 order: [dxx0, dyy0, dxx1, dyy1, dxx2, dyy2]
DIDX = {('x', 0): 0, ('y', 0): 1, ('x', 1): 2, ('y', 1): 3,
        ('x', 2): 4, ('y', 2): 5}


def _eps_schedule():
    eps_list = []
    e = 1.0
    while e > EPS_FINAL:
        eps_list.append(e)
        e = e * 0.25
    eps_list.append(EPS_FINAL)
    return eps_list


def build_bass():
    import concourse.bass as bass
    import concourse.bacc as bacc
    import concourse.tile as tile
    from concourse import mybir
    from concourse.masks import make_identity

    f32 = mybir.dt.float32
    bf16 = mybir.dt.bfloat16
    i32 = mybir.dt.int32
    Alu = mybir.AluOpType
    Act = mybir.ActivationFunctionType
    X = mybir.AxisListType.X

    nc = bacc.Bacc(
        "TRN2",
        target_bir_lowering=False,
        debug=False,
        num_devices=NCORES,
    )

    xs = [nc.declare_dram_parameter(n, [B, QS, T], bf16, isOutput=False)
          for n in ("xc", "xt", "xe")]
    ys = [nc.declare_dram_parameter(n, [B, QS, T], bf16, isOutput=False)
          for n in ("yc", "yt", "ye")]
    dbc = nc.declare_dram_parameter("dbc", [B, QS, S], bf16, isOutput=False)
    dbn = nc.declare_dram_parameter("dbn", [B, QS, S], bf16, isOutput=False)
    ehs = nc.declare_dram_parameter("ehs", [B, EPAD, H], bf16, isOutput=False)
    eht = nc.declare_dram_parameter("eht", [B, EPAD, H], bf16, isOutput=False)
    eds = nc.declare_dram_parameter("eds", [B, EPAD, H], bf16, isOutput=False)
    edt = nc.declare_dram_parameter("edt", [B, EPAD, H], bf16, isOutput=False)
    auxf = nc.declare_dram_parameter("auxf", [B, 14], f32, isOutput=False)
    auxb = nc.declare_dram_parameter("auxb", [B, 28], bf16, isOutput=False)
    auxi = nc.declare_dram_parameter("auxi", [B, 2], i32, isOutput=False)
    out_ext = nc.declare_dram_parameter("out", [1, 1], f32, isOutput=True)

    c1_in = nc.dram_tensor("c1_in", [C1ROWS, 128], bf16)
    c1_out = nc.dram_tensor("c1_out", [C1ROWS, 128], bf16, addr_space="Shared")
    c2_in = nc.dram_tensor("c2_in", [C2ROWS, 128], bf16)
    c2_out = nc.dram_tensor("c2_out", [C2ROWS, 128], bf16, addr_space="Shared")

    # constants baked into the NEFF
    import ml_dtypes
    msk_np = np.zeros((2, 256), np.float32)
    msk_np[0, 0:128] = 1.0
    msk_np[1, 128:256] = 1.0
    msk_dram = nc.inline_tensor(msk_np.astype(ml_dtypes.bfloat16), "mskc")
    idx_np = np.broadcast_to(np.arange(64, dtype=np.float32), (B, 64)).copy()
    idx_dram = nc.inline_tensor(idx_np, "idxc")

    blog = float(-np.log(float(B)))
    groups = [list(range(NCORES))]

    with nc.allow_low_precision(reason="bf16 partial sums are exact or slack-validated"), \
         tile.TileContext(nc) as tc:
        with tc.tile_pool(name="persist", bufs=1) as persist:
            identf = persist.tile([128, 128], f32)
            make_identity(nc, identf[:])
            identb = persist.tile([128, 128], bf16)
            nc.vector.tensor_copy(identb[:], identf[:])

            cesb = persist.tile([B, 256], bf16)
            nc.vector.memset(cesb[:], 0.0)
            cesb2 = persist.tile([B, 128], bf16)
            nc.vector.memset(cesb2[:], 0.0)
            delta = persist.tile([B, QS, S], bf16)

            # ---------------- phase A ----------------
            with (
                tc.tile_pool(name="sload", bufs=2) as sload,
                tc.tile_pool(name="bload", bufs=1) as bload,
                tc.tile_pool(name="tpool", bufs=2) as tpool,
                tc.tile_pool(name="mpool", bufs=2) as mpool,
                tc.tile_pool(name="epool", bufs=2) as epool,
                tc.tile_pool(name="gevac", bufs=2) as gevac,
                tc.tile_pool(name="gpsum", bufs=2, space="PSUM") as gpsum,
            ):
                # batch: delta + a partials
                bct = bload.tile([B, QS, S], bf16, tag="bc")
                nc.sync.dma_start(out=bct[:], in_=dbc[:, :, :])
                bnt = bload.tile([B, QS, S], bf16, tag="bn")
                nc.sync.dma_start(out=bnt[:], in_=dbn[:, :, :])
                nc.vector.tensor_add(delta[:], bct[:], bnt[:])
                dif = bload.tile([B, QS, S], bf16, tag="dif")
                nc.vector.tensor_sub(dif[:], bct[:], bnt[:])
                nc.vector.reduce_sum(
                    out=cesb[:, 128:128 + S],
                    in_=dif[:].rearrange("b q s -> b s q"), axis=X)

                # embed partials
                ecols = persist.tile([B, 2], f32)
                for k, (ea, eb) in enumerate(((ehs, eht), (eds, edt))):
                    e1 = epool.tile([B, EPAD * H], bf16, tag="ea")
                    nc.scalar.dma_start(out=e1[:],
                                        in_=ea[:].rearrange("b t h -> b (t h)"))
                    e2 = epool.tile([B, EPAD * H], bf16, tag="eb")
                    nc.scalar.dma_start(out=e2[:],
                                        in_=eb[:].rearrange("b t h -> b (t h)"))
                    ed = epool.tile([B, EPAD * H], bf16, tag="ed")
                    nc.vector.tensor_sub(ed[:], e1[:], e2[:])
                    esq = epool.tile([B, EPAD * H], bf16, tag="esq")
                    nc.scalar.activation(esq[:], ed[:], Act.Square,
                                         accum_out=ecols[:, k:k + 1])
                embf = persist.tile([B, 1], f32)
                nc.vector.tensor_add(embf[:], ecols[:, 0:1], ecols[:, 1:2])
                nc.vector.tensor_copy(cesb[:, 192:193], embf[:])

                # grams + CE gathers per pair
                for p in range(3):
                    xsb = sload.tile([B, QS, T], bf16, tag="xs")
                    nc.sync.dma_start(out=xsb[:], in_=xs[p][:, :, :])
                    tp = tpool.tile([128, 2, T, 128], bf16, tag="tp")
                    nc.scalar.dma_start(
                        out=tp[:, 0, :, :],
                        in_=xsb[:].rearrange("b q t -> b (q t)"),
                        transpose=True)
                    nc.scalar.dma_start(
                        out=tp[:, 1, :, :],
                        in_=ys[p][:, :, :].rearrange("b q t -> b (q t)"),
                        transpose=True)

                    gpa = gpsum.tile([128, 256], f32, tag="ga")
                    gyy = gpsum.tile([128, 128], f32, tag="gy")
                    for t in range(T):
                        nc.tensor.matmul(
                            gpa[:], tp[:, 0, t, :], tp[:, :, t, :],
                            start=(t == 0), stop=(t == T - 1))
                        nc.tensor.matmul(
                            gyy[:], tp[:, 1, t, :], tp[:, 1, t, :],
                            start=(t == 0), stop=(t == T - 1))

                    # CE gather for this (student) pair
                    ms = mpool.tile([B, QS, S], bf16, tag="ms")
                    nc.vector.tensor_mul(ms[:], xsb[:, :, 0:S], delta[:])
                    pcdst = (cesb[:, 64 * p:64 * p + S] if p < 2
                             else cesb2[:, 0:S])
                    nc.vector.reduce_sum(
                        out=pcdst, in_=ms[:].rearrange("b q s -> b s q"), axis=X)

                    # evacuate grams (bf16) + diag partials
                    gsb = gevac.tile([B, 384], bf16, tag="gsb")
                    nc.scalar.copy(gsb[:, 0:256], gpa[:])
                    nc.vector.tensor_copy(gsb[:, 256:384], gyy[:])
                    dsc = mpool.tile([B, 128], bf16, tag="dsc")
                    ddst = (cesb[:, 193 + 2 * p:195 + 2 * p] if p < 2
                            else cesb2[:, 64:66])
                    nc.vector.tensor_mul(dsc[:], gsb[:, 0:128], identb[:])
                    nc.vector.reduce_sum(out=ddst[:, 0:1], in_=dsc[:], axis=X)
                    dsc2 = mpool.tile([B, 128], bf16, tag="dsc")
                    nc.vector.tensor_mul(dsc2[:], gsb[:, 256:384], identb[:])
                    nc.vector.reduce_sum(out=ddst[:, 1:2], in_=dsc2[:], axis=X)

                    # stage mat blocks to collective input buffers
                    if p < 2:
                        xyr, xxr, yyr = 128 * p, 128 * (2 + p), 128 * (4 + p)
                        nc.sync.dma_start(out=c1_in[xyr:xyr + 128, :],
                                          in_=gsb[:, 128:256])
                        nc.sync.dma_start(out=c1_in[xxr:xxr + 128, :],
                                          in_=gsb[:, 0:128])
                        nc.sync.dma_start(out=c1_in[yyr:yyr + 128, :],
                                          in_=gsb[:, 256:384])
                    else:
                        nc.sync.dma_start(out=c2_in[0:128, :],
                                          in_=gsb[:, 128:256])
                        nc.sync.dma_start(out=c2_in[128:256, :],
                                          in_=gsb[:, 0:128])
                        nc.sync.dma_start(out=c2_in[256:384, :],
                                          in_=gsb[:, 256:384])

                    if p == 1:
                        # CE-extra block rides C1; fire C1 while pair 2 runs
                        nc.sync.dma_start(out=c1_in[768:896, :],
                                          in_=cesb[:, 0:128])
                        nc.sync.dma_start(out=c1_in[896:1024, :],
                                          in_=cesb[:, 128:256])
                        nc.gpsimd.collective_compute(
                            "AllReduce", Alu.add, replica_groups=groups,
                            ins=[c1_in[:, :]], outs=[c1_out[:, :]])
                nc.sync.dma_start(out=c2_in[384:512, :], in_=cesb2[:])
                nc.gpsimd.collective_compute(
                    "AllReduce", Alu.add, replica_groups=groups,
                    ins=[c2_in[:, :]], outs=[c2_out[:, :]])

            # ---------------- phase B ----------------
            with (
                tc.tile_pool(name="pbig", bufs=2) as pbig,
                tc.tile_pool(name="psmall", bufs=2) as psmall,
                tc.tile_pool(name="pconst", bufs=1) as pconst,
                tc.tile_pool(name="hps", bufs=2, space="PSUM") as hpsum,
                tc.tile_pool(name="fps", bufs=2, space="PSUM") as fpsum,
                tc.tile_pool(name="sps", bufs=1, space="PSUM") as spsum,
            ):
                # aux inputs
                axf = pconst.tile([B, 14], f32)
                nc.scalar.dma_start(out=axf[:], in_=auxf[:, :])
                axb = pconst.tile([B, 28], bf16)
                nc.scalar.dma_start(out=axb[:], in_=auxb[:, :])
                axi = pconst.tile([B, 2], i32)
                nc.scalar.dma_start(out=axi[:], in_=auxi[:, :])
                mskt = pconst.tile([2, 256], bf16)
                nc.scalar.dma_start(out=mskt[:], in_=msk_dram[:, :])
                ones2b = pconst.tile([2, 128], bf16)
                nc.vector.memset(ones2b[:], 1.0)
                ones_col = pconst.tile([B, 1], f32)
                nc.vector.memset(ones_col[:], 1.0)

                # CE-extra replicated blocks
                c1post = pconst.tile([B, 256], bf16)
                nc.sync.dma_start(out=c1post[:, 0:128], in_=c1_out[768:896, :])
                nc.sync.dma_start(out=c1post[:, 128:256], in_=c1_out[896:1024, :])
                c2post = pconst.tile([B, 128], bf16)
                nc.sync.dma_start(out=c2post[:], in_=c2_out[384:512, :])

                # slot gathers (per-core indices)
                r0 = pconst.tile([B, 128], bf16)
                nc.gpsimd.indirect_dma_start(
                    out=r0[:], out_offset=None,
                    in_=c1_out[0:768, :],
                    in_offset=bass.IndirectOffsetOnAxis(ap=axi[:, 0:1], axis=0))
                r1 = pconst.tile([B, 128], bf16)
                nc.gpsimd.indirect_dma_start(
                    out=r1[:], out_offset=None,
                    in_=c2_out[0:384, :],
                    in_offset=bass.IndirectOffsetOnAxis(ap=axi[:, 1:2], axis=0))
                t0p = fpsum.tile([128, 128], bf16, tag="t0")
                nc.tensor.transpose(t0p[:], r0[:], identb[:])
                t0 = pconst.tile([B, 128], bf16)
                nc.vector.tensor_copy(t0[:], t0p[:])
                t1p = fpsum.tile([128, 128], bf16, tag="t0")
                nc.tensor.transpose(t1p[:], r1[:], identb[:])
                t1 = pconst.tile([B, 128], bf16)
                nc.vector.tensor_copy(t1[:], t1p[:])
                # Gsl[:,0,:] = a0*r0 + (1-a0)*t1 ; Gsl[:,1,:] = a1*r1 + (1-a1)*t0
                Gsl = pconst.tile([B, 2, 128], bf16)
                gtmp = psmall.tile([B, 128], bf16, tag="gtmp")
                nc.vector.tensor_scalar(gtmp[:], t1[:], axf[:, 11:12], None,
                                        Alu.mult)
                nc.vector.scalar_tensor_tensor(Gsl[:, 0, :], r0[:], axf[:, 10:11],
                                               gtmp[:], Alu.mult, Alu.add)
                gtmp2 = psmall.tile([B, 128], bf16, tag="gtmp")
                nc.vector.tensor_scalar(gtmp2[:], t0[:], axf[:, 13:14], None,
                                        Alu.mult)
                nc.vector.scalar_tensor_tensor(Gsl[:, 1, :], r1[:], axf[:, 12:13],
                                               gtmp2[:], Alu.mult, Alu.add)

                # diag6 + per-slot D2 / DH via shipped selection masks
                diag6 = pconst.tile([B, 6], bf16)
                nc.vector.tensor_copy(diag6[:, 0:4], c1post[:, 193:197])
                nc.vector.tensor_copy(diag6[:, 4:6], c2post[:, 64:66])
                D2 = pconst.tile([B, 2], f32)
                DH = pconst.tile([B, 2], f32)
                for s in range(2):
                    selr = axb[:, 4 + 6 * s:10 + 6 * s]
                    selc = axb[:, 16 + 6 * s:22 + 6 * s]
                    dt_ = psmall.tile([B, 6], bf16, tag="dt")
                    nc.vector.tensor_mul(dt_[:], diag6[:], selr)
                    nc.vector.reduce_sum(out=D2[:, s:s + 1], in_=dt_[:], axis=X)
                    dt2 = psmall.tile([B, 6], bf16, tag="dt")
                    nc.vector.tensor_mul(dt2[:], diag6[:], selc)
                    nc.vector.reduce_sum(out=DH[:, s:s + 1], in_=dt2[:], axis=X)

                F = pconst.tile([B, 2], f32)
                nc.vector.memset(F[:], 0.0)
                sv = pconst.tile([B, 2], f32)
                lg = pconst.tile([B, 2], f32)
                mv = pconst.tile([B, 2], f32)
                scrx = pconst.tile([B, 2, 128], bf16)
                escr = pconst.tile([B, 128], bf16)

                for eps in _eps_schedule():
                    damp = 1.0 / (1.0 + eps / RHO)
                    c = GSCALE / eps
                    fsum = psmall.tile([B, 2], f32, tag="fsum")
                    nc.vector.tensor_add(fsum[:], F[:], DH[:])
                    ftp = fpsum.tile([2, 128], f32, tag="ft")
                    nc.tensor.transpose(ftp[:], fsum[:], identf[:])
                    HT = psmall.tile([2, 128], bf16, tag="ht")
                    nc.vector.tensor_scalar(HT[:], ftp[:], 1.0 / GSCALE,
                                            blog * eps / GSCALE,
                                            Alu.mult, Alu.add)
                    rhm = psmall.tile([2, 2, 128], bf16, tag="rhm")
                    nc.vector.tensor_tensor(
                        rhm[:], HT[:].unsqueeze(1).broadcast_to((2, 2, 128)),
                        mskt[:].rearrange("k (a j) -> k a j", j=128), Alu.mult)
                    hbt = hpsum.tile([128, 256], f32, tag="hb")
                    nc.tensor.matmul(hbt[:], ones2b[:],
                                     rhm[:].rearrange("k a j -> k (a j)"),
                                     start=True, stop=False)
                    nc.tensor.matmul(hbt[:], identb[:],
                                     Gsl[:].rearrange("b a j -> b (a j)"),
                                     start=False, stop=True)
                    hb3 = hbt[:].rearrange("b (s j) -> b s j", j=128)
                    nc.vector.reduce_max(out=mv[:], in_=hb3, axis=X)
                    nc.vector.tensor_tensor(
                        scrx[:], hb3,
                        mv[:].unsqueeze(2).broadcast_to((B, 2, 128)),
                        Alu.subtract)
                    for s in range(2):
                        nc.scalar.activation(escr[:], scrx[:, s, :], Act.Exp,
                                             scale=float(c),
                                             accum_out=sv[:, s:s + 1])
                    nc.scalar.activation(lg[:], sv[:], Act.Ln)
                    # dmu = D2 - eps*lg - GSCALE*mv
                    dm1 = psmall.tile([B, 2], f32, tag="dm1")
                    nc.vector.scalar_tensor_tensor(dm1[:], lg[:], float(-eps),
                                                   D2[:], Alu.mult, Alu.add)
                    dmu = psmall.tile([B, 2], f32, tag="dmu")
                    nc.vector.scalar_tensor_tensor(dmu[:], mv[:], float(-GSCALE),
                                                   dm1[:], Alu.mult, Alu.add)
                    dr = psmall.tile([B, 2], f32, tag="dr")
                    nc.vector.tensor_copy(dr[:, 0:1], dmu[:, 1:2])
                    nc.vector.tensor_copy(dr[:, 1:2], dmu[:, 0:1])
                    # cmix = damp * ((1-pf)*dmu + pf*rev(dmu))
                    c1t = psmall.tile([B, 2], f32, tag="c1t")
                    nc.vector.tensor_scalar(c1t[:], dmu[:], axf[:, 1:2],
                                            float(damp), Alu.mult, Alu.mult)
                    c2t = psmall.tile([B, 2], f32, tag="c2t")
                    nc.vector.tensor_scalar(c2t[:], dr[:], axf[:, 0:1],
                                            float(damp), Alu.mult, Alu.mult)
                    cmix = psmall.tile([B, 2], f32, tag="cmix")
                    nc.vector.tensor_add(cmix[:], c1t[:], c2t[:])
                    # F = wF*F + vF*cmix
                    m1 = psmall.tile([B, 2], f32, tag="m1")
                    nc.vector.tensor_mul(m1[:], F[:], axf[:, 2:4])
                    m2 = psmall.tile([B, 2], f32, tag="m2")
                    nc.vector.tensor_mul(m2[:], cmix[:], axf[:, 4:6])
                    nc.vector.tensor_add(F[:], m1[:], m2[:])

                # ---- loss_kd partial ----
                E2 = psmall.tile([B, 2], f32, tag="e2")
                nc.scalar.activation(E2[:], F[:], Act.Exp, scale=float(-1.0 / RHO))
                km = psmall.tile([B, 2], f32, tag="km")
                nc.vector.tensor_mul(km[:], E2[:], axf[:, 6:8])
                kdp = psmall.tile([B, 1], f32, tag="kdp")
                nc.vector.reduce_sum(out=kdp[:], in_=km[:], axis=X)

                # ---- CE (replicated; gated by aux csup/cemb) ----
                pcall = pconst.tile([B, 192], f32)
                nc.vector.tensor_copy(pcall[:, 0:128], c1post[:, 0:128])
                nc.vector.tensor_copy(pcall[:, 128:192], c2post[:, 0:64])
                af = pconst.tile([B, 64], f32)
                nc.vector.tensor_copy(af[:], c1post[:, 128:192])
                embcol = pconst.tile([B, 1], f32)
                nc.vector.tensor_copy(embcol[:], c1post[:, 192:193])

                idxf = pconst.tile([B, 64], f32)
                nc.scalar.dma_start(out=idxf[:], in_=idx_dram[:, :])
                pos = psmall.tile([B, 64], f32, tag="pos")
                nc.vector.tensor_scalar(pos[:], pcall[:, 0:64], 0.0, None,
                                        Alu.is_gt)
                ip1 = psmall.tile([B, 64], f32, tag="ip1")
                nc.vector.scalar_tensor_tensor(ip1[:], idxf[:], 1.0, pos[:],
                                               Alu.add, Alu.mult)
                Lp = psmall.tile([B, 1], f32, tag="Lp")
                nc.vector.reduce_max(out=Lp[:], in_=ip1[:], axis=X)
                eq0 = psmall.tile([B, 1], f32, tag="eq0")
                nc.vector.tensor_scalar(eq0[:], Lp[:], 0.0, None, Alu.is_equal)
                Lv = psmall.tile([B, 1], f32, tag="Lv")
                nc.vector.scalar_tensor_tensor(Lv[:], eq0[:], float(S), Lp[:],
                                               Alu.mult, Alu.add)
                dl = psmall.tile([B, 64], f32, tag="dl")
                nc.vector.tensor_scalar(dl[:], idxf[:], Lv[:, 0:1], None,
                                        Alu.subtract)
                mask = psmall.tile([B, 64], f32, tag="mask")
                nc.vector.tensor_scalar(mask[:], dl[:], 0.0, None, Alu.is_lt)
                negf = psmall.tile([B, 64], f32, tag="negf")
                nc.vector.tensor_scalar(negf[:], mask[:], 1.0, 1e9,
                                        Alu.subtract, Alu.mult)
                # a = floor((asum+1)/2) via magic round (values < 2^22)
                MAGIC = 12582912.0
                tv = psmall.tile([B, 64], f32, tag="tv")
                nc.vector.tensor_scalar(tv[:], af[:], 0.5, 1024.25,
                                        Alu.mult, Alu.add)
                tm = psmall.tile([B, 64], f32, tag="tm")
                nc.vector.tensor_scalar(tm[:], tv[:], MAGIC, MAGIC,
                                        Alu.add, Alu.subtract)
                av = psmall.tile([B, 64], f32, tag="av")
                nc.vector.tensor_scalar(av[:], tm[:], 1024.0, None, Alu.subtract)
                amask = psmall.tile([B, 64], f32, tag="amask")
                nc.vector.tensor_tensor(amask[:], av[:], mask[:], Alu.mult)
                pc3 = pcall[:].rearrange("b (s q) -> b s q", q=64)
                mce = pbig.tile([B, 3, 64], f32, tag="mce")
                mask3 = mask[:].unsqueeze(1).broadcast_to((B, 3, 64))
                negf3 = negf[:].unsqueeze(1).broadcast_to((B, 3, 64))
                amask3 = amask[:].unsqueeze(1).broadcast_to((B, 3, 64))
                t2_ = pbig.tile([B, 3, 64], f32, tag="tt")
                nc.vector.scalar_tensor_tensor(t2_[:], pc3, 2.0, mask3, Alu.mult,
                                               Alu.mult)
                nc.vector.tensor_tensor(mce[:], t2_[:], negf3, Alu.add)
                mx3 = psmall.tile([B, 3], f32, tag="mx3")
                nc.vector.reduce_max(out=mx3[:], in_=mce[:], axis=X)
                mb3 = mx3[:].unsqueeze(2).broadcast_to((B, 3, 64))
                dd = pbig.tile([B, 3, 64], f32, tag="dd")
                nc.vector.tensor_tensor(dd[:], mce[:], mb3, Alu.subtract)
                ee = pbig.tile([B, 3, 64], f32, tag="ee")
                nc.scalar.activation(ee[:], dd[:], Act.Exp)
                ss3 = psmall.tile([B, 3], f32, tag="ss3")
                nc.vector.reduce_sum(out=ss3[:], in_=ee[:], axis=X)
                lg3 = psmall.tile([B, 3], f32, tag="lg3")
                nc.scalar.activation(lg3[:], ss3[:], Act.Ln)
                lse3 = psmall.tile([B, 3], f32, tag="lse3")
                nc.vector.tensor_add(lse3[:], mx3[:], lg3[:])
                lb3 = lse3[:].unsqueeze(2).broadcast_to((B, 3, 64))
                d1 = pbig.tile([B, 3, 64], f32, tag="dd")
                nc.vector.tensor_tensor(d1[:], mce[:], lb3, Alu.subtract)
                d2_ = pbig.tile([B, 3, 64], f32, tag="tt")
                nc.vector.tensor_tensor(d2_[:], d1[:], amask3, Alu.mult)
                rowsum = psmall.tile([B, 1], f32, tag="rs")
                nc.vector.reduce_sum(out=rowsum[:],
                                     in_=d2_[:].rearrange("b s q -> b (s q)"),
                                     axis=X)

                # ---- final combine: csup*CE + cemb*embed + kd_partial ----
                tot_ps = spsum.tile([1, 1], f32, tag="tot")
                nc.tensor.matmul(tot_ps[:], rowsum[:], axf[:, 8:9], start=True,
                                 stop=False)
                nc.tensor.matmul(tot_ps[:], embcol[:], axf[:, 9:10], start=False,
                                 stop=False)
                nc.tensor.matmul(tot_ps[:], kdp[:], ones_col[:], start=False,
                                 stop=True)
                outt = psmall.tile([1, 1], f32, tag="outt")
                nc.vector.tensor_copy(outt[:], tot_ps[:])
                nc.sync.dma_start(out=out_ext[:, :], in_=outt[:])

    nc.compile()
    return nc


_NC = None
LAST_RESULTS = None


def _core_aux(c):
    sl = SLOTS[c]
    i0 = C1MAT[sl['i0']] if sl['i0'] is not None else 0
    i1 = C2MAT[sl['i1']] if sl['i1'] is not None else 0
    pf = float(sl['pf'])
    wf = [0.0, 0.0] if sl['pf'] else [0.5, 0.5]
    vf = [1.0, 1.0] if sl['pf'] else [0.5, 0.5]
    cgate = 1.0 if c == 0 else 0.0
    auxf = np.zeros((B, 14), np.float32)
    auxf[:, 0] = pf
    auxf[:, 1] = 1.0 - pf
    auxf[:, 2:4] = wf
    auxf[:, 4:6] = vf
    auxf[:, 6:8] = sl['kc']
    auxf[:, 8] = -LOSS_WEIGHT * SUP_W * cgate
    auxf[:, 9] = LOSS_WEIGHT * EMBED_W * 0.5 * cgate
    auxf[:, 10] = float(sl['a0'])
    auxf[:, 11] = 1.0 - float(sl['a0'])
    auxf[:, 12] = float(sl['a1'])
    auxf[:, 13] = 1.0 - float(sl['a1'])
    auxb = np.zeros((B, 28), np.float32)
    auxb[:, 0] = float(sl['a0'])
    auxb[:, 1] = 1.0 - float(sl['a0'])
    auxb[:, 2] = float(sl['a1'])
    auxb[:, 3] = 1.0 - float(sl['a1'])
    for s in range(2):
        auxb[:, 4 + 6 * s + DIDX[sl['rs'][s]]] = 2.0
        auxb[:, 16 + 6 * s + DIDX[sl['cs'][s]]] = -2.0
    auxi = np.zeros((B, 2), np.int32)
    auxi[:, 0] = 128 * i0 + np.arange(B)
    auxi[:, 1] = 128 * i1 + np.arange(B)
    return auxf, auxb, auxi


def _shard_inputs(logit_c, logit_t, logit_ensemble, logit_teacher_c,
                  logit_teacher_t, logit_teacher_ensemble, out_h_student,
                  out_h_teacher, out_d_student, out_d_teacher, batch):
    import ml_dtypes
    bf = ml_dtypes.bfloat16
    students = [logit_c, logit_t, logit_ensemble]
    teachers = [logit_teacher_c, logit_teacher_t, logit_teacher_ensemble]
    embeds = dict(ehs=out_h_student, eht=out_h_teacher,
                  eds=out_d_student, edt=out_d_teacher)
    # q-major [B, QS, T] bf16 per core
    sb = [np.ascontiguousarray(np.transpose(
        np.asarray(a, np.float32).astype(bf), (0, 2, 1))) for a in students]
    tb = [np.ascontiguousarray(np.transpose(
        np.asarray(a, np.float32).astype(bf), (0, 2, 1))) for a in teachers]
    bcq = np.ascontiguousarray(np.transpose(
        np.asarray(batch[:, 1:1 + S, :Q], np.float32).astype(bf), (0, 2, 1)))
    bnq = np.ascontiguousarray(np.transpose(
        np.asarray(batch[:, 1:1 + S, Q:], np.float32).astype(bf), (0, 2, 1)))
    in_maps = []
    for c in range(NCORES):
        q0 = QS * c
        m = {}
        for nm, arr in zip(("xc", "xt", "xe"), sb):
            m[nm] = np.ascontiguousarray(arr[:, q0:q0 + QS, :])
        for nm, arr in zip(("yc", "yt", "ye"), tb):
            m[nm] = np.ascontiguousarray(arr[:, q0:q0 + QS, :])
        m["dbc"] = np.ascontiguousarray(bcq[:, q0:q0 + QS, :])
        m["dbn"] = np.ascontiguousarray(bnq[:, q0:q0 + QS, :])
        t0, w = EOFF[c], ESPLIT[c]
        for nm, arr in embeds.items():
            sl = np.zeros((B, EPAD, H), bf)
            sl[:, :w, :] = np.asarray(arr[:, t0:t0 + w, :], np.float32).astype(bf)
            m[nm] = sl
        axf, axb, axi = _core_aux(c)
        m["auxf"] = axf
        m["auxb"] = axb.astype(bf)
        m["auxi"] = axi
        in_maps.append(m)
    return in_maps


def kernel(**inputs):
    global _NC, LAST_RESULTS
    from concourse.bass_utils import run_bass_kernel_spmd
    if _NC is None:
        _NC = build_bass()
    in_maps = _shard_inputs(**inputs)
    trace = bool(int(os.environ.get("KERNEL_TRACE", "0")))
    res = run_bass_kernel_spmd(_NC, in_maps, list(range(NCORES)), trace=trace)
    LAST_RESULTS = res
    total = sum(float(np.asarray(r["out"]).reshape(-1)[0]) for r in res.results)
    return np.asarray([total], dtype=np.float32)


# revision 16
# speedup vs baseline: 1.4634x; 1.0276x over previous
"""Trainium2 Bass kernel for nn_CombinedLoss (sinkhorn-KD + soft-CE + embed MSE).

v2 architecture (8 cores):
  - All inputs shipped bf16. Logits in q-major [B, QS, T] per-core layout so a
    single XBAR DMA-transpose per tensor yields [q, t, b] gram operands
    (no PE transposes, no PSUM evacuation of transposed data).
  - Grams per pair: [G_xx | G_xy] and G_yy only; G_yx derived on the owner
    core as G_xy^T after reduction.
  - Two bf16 AllReduces in [mat*128+b, j] block layout: C1 (pairs 0/1 grams +
    CE/a/embed/diag cols) fires after pair 1 and hides under pair-2 compute;
    C2 (pair-2 grams + pc2/diag2) is the only exposed collective.
  - Phase B (9 sinkhorn iterations) sharded: each core processes only 2 of
    the 12 B x B matrices, selected via per-core indirect-DMA row gathers and
    data-driven update rules (uniform SPMD program).
  - Final loss_kd partials stay per-core; the host sums the 8 scalar outputs
    (the unshard step - all loss reductions are sums over shards).
"""
import os
import numpy as np

B = 128
T = 50
Q = 1024
S = 49          # MAX_STEP - 1
H = 256
NCORES = 8
QS = Q // NCORES
TEMP = 0.5
GSCALE = 1.0 / (TEMP * TEMP)   # p-gram = GSCALE * logit-gram = 4
RHO = 500.0 ** 2
EPS_FINAL = 0.005 ** 2
SUP_W, DIST_W, EMBED_W, LOSS_WEIGHT = 1.0, 0.01, 1.0, 1.0
CKD = float(LOSS_WEIGHT * DIST_W * (RHO + EPS_FINAL / 2.0) / B)

# embed t-shard split (padded to 7 per core)
ESPLIT = [7, 7, 6, 6, 6, 6, 6, 6]
EOFF = [0, 7, 14, 20, 26, 32, 38, 44]
EPAD = 7

# c1 AllReduce buffer: [1024, 128] bf16 rows; mat k occupies rows 128k..128k+127
# mats: xy0=0, xy1=1, xx0=2, xx1=3, yy0=4, yy1=5; CE-extra blocks rows 768-1023
# CE-extra cols (within the logical [B, 256] block):
#   pc0 at 0-63, pc1 64-127, a 128-191, embed 192, diag01 193-196
C1ROWS = 1024
# c2: xy2=0, xx2=1, yy2=2; CE2-extra rows 384-511: pc2 0-63, diag2 64-65
C2ROWS = 512

C1MAT = {('xy', 0): 0, ('xy', 1): 1, ('xx', 0): 2, ('xx', 1): 3,
         ('yy', 0): 4, ('yy', 1): 5}
C2MAT = {('xy', 2): 0, ('xx', 2): 1, ('yy', 2): 2}

# per-core slot assignment: (slot0 src, slot1 src); None = derived/dead
# a0=1: slot0 <- c1 read; a0=0: slot0 <- transpose(c2 read). etc.
SLOTS = {
    0: dict(i0=('xy', 0), i1=None, a0=1, a1=0, pf=1,
            kc=[-CKD, -CKD], rs=[('x', 0), ('y', 0)], cs=[('y', 0), ('x', 0)]),
    1: dict(i0=('xy', 1), i1=None, a0=1, a1=0, pf=1,
            kc=[-CKD, -CKD], rs=[('x', 1), ('y', 1)], cs=[('y', 1), ('x', 1)]),
    2: dict(i0=None, i1=('xy', 2), a0=0, a1=1, pf=1,
            kc=[-CKD, -CKD], rs=[('y', 2), ('x', 2)], cs=[('x', 2), ('y', 2)]),
    3: dict(i0=('xx', 0), i1=('xy', 2), a0=1, a1=1, pf=0,
            kc=[CKD, 0.0], rs=[('x', 0), ('x', 2)], cs=[('x', 0), ('y', 2)]),
    4: dict(i0=('xx', 1), i1=('xy', 2), a0=1, a1=1, pf=0,
            kc=[CKD, 0.0], rs=[('x', 1), ('x', 2)], cs=[('x', 1), ('y', 2)]),
    5: dict(i0=('xy', 0), i1=('xx', 2), a0=1, a1=1, pf=0,
            kc=[0.0, CKD], rs=[('x', 0), ('x', 2)], cs=[('y', 0), ('x', 2)]),
    6: dict(i0=('yy', 0), i1=('yy', 2), a0=1, a1=1, pf=0,
            kc=[CKD, CKD], rs=[('y', 0), ('y', 2)], cs=[('y', 0), ('y', 2)]),
    7: dict(i0=('yy', 1), i1=('xy', 2), a0=1, a1=1, pf=0,
            kc=[CKD, 0.0], rs=[('y', 1), ('y', 2)], cs=[('y', 1), ('y', 2)]),
}
# diag6 order: [dxx0, dyy0, dxx1, dyy1, dxx2, dyy2]
DIDX = {('x', 0): 0, ('y', 0): 1, ('x', 1): 2, ('y', 1): 3,
        ('x', 2): 4, ('y', 2): 5}


def _eps_schedule():
    eps_list = []
    e = 1.0
    while e > EPS_FINAL:
        eps_list.append(e)
        e = e * 0.25
    eps_list.append(EPS_FINAL)
    return eps_list


def build_bass():
    import concourse.bass as bass
    import concourse.bacc as bacc
    import concourse.tile as tile
    from concourse import mybir
    from concourse.masks import make_identity

    f32 = mybir.dt.float32
    bf16 = mybir.dt.bfloat16
    i32 = mybir.dt.int32
    Alu = mybir.AluOpType
    Act = mybir.ActivationFunctionType
    X = mybir.AxisListType.X

    nc = bacc.Bacc(
        "TRN2",
        target_bir_lowering=False,
        debug=False,
        num_devices=NCORES,
    )

    xs = [nc.declare_dram_parameter(n, [B, QS, T], bf16, isOutput=False)
          for n in ("xc", "xt", "xe")]
    ys = [nc.declare_dram_parameter(n, [B, QS, T], bf16, isOutput=False)
          for n in ("yc", "yt", "ye")]
    xst = [nc.declare_dram_parameter(n, [B, S, QS], bf16, isOutput=False)
           for n in ("xct", "xtt", "xet")]
    dbc = nc.declare_dram_parameter("dbc", [B, S, QS], bf16, isOutput=False)
    dbn = nc.declare_dram_parameter("dbn", [B, S, QS], bf16, isOutput=False)
    ehs = nc.declare_dram_parameter("ehs", [B, EPAD, H], bf16, isOutput=False)
    eht = nc.declare_dram_parameter("eht", [B, EPAD, H], bf16, isOutput=False)
    eds = nc.declare_dram_parameter("eds", [B, EPAD, H], bf16, isOutput=False)
    edt = nc.declare_dram_parameter("edt", [B, EPAD, H], bf16, isOutput=False)
    auxf = nc.declare_dram_parameter("auxf", [B, 14], f32, isOutput=False)
    auxb = nc.declare_dram_parameter("auxb", [B, 28], bf16, isOutput=False)
    auxi = nc.declare_dram_parameter("auxi", [B, 2], i32, isOutput=False)
    out_ext = nc.declare_dram_parameter("out", [1, 1], f32, isOutput=True)

    c1_in = nc.dram_tensor("c1_in", [C1ROWS, 128], bf16)
    c1_out = nc.dram_tensor("c1_out", [C1ROWS, 128], bf16, addr_space="Shared")
    c2_in = nc.dram_tensor("c2_in", [C2ROWS, 128], bf16)
    c2_out = nc.dram_tensor("c2_out", [C2ROWS, 128], bf16, addr_space="Shared")

    # constants baked into the NEFF
    import ml_dtypes
    msk_np = np.zeros((2, 256), np.float32)
    msk_np[0, 0:128] = 1.0
    msk_np[1, 128:256] = 1.0
    msk_dram = nc.inline_tensor(msk_np.astype(ml_dtypes.bfloat16), "mskc")
    idx_np = np.broadcast_to(np.arange(64, dtype=np.float32), (B, 64)).copy()
    idx_dram = nc.inline_tensor(idx_np, "idxc")

    blog = float(-np.log(float(B)))
    groups = [list(range(NCORES))]

    with nc.allow_low_precision(reason="bf16 partial sums are exact or slack-validated"), \
         tile.TileContext(nc) as tc:
        with tc.tile_pool(name="persist", bufs=1) as persist:
            identf = persist.tile([128, 128], f32)
            make_identity(nc, identf[:])
            identb = persist.tile([128, 128], bf16)
            nc.vector.tensor_copy(identb[:], identf[:])

            cesb = persist.tile([B, 256], bf16)
            nc.vector.memset(cesb[:], 0.0)
            cesb2 = persist.tile([B, 128], bf16)
            nc.vector.memset(cesb2[:], 0.0)
            delta = persist.tile([B, S, QS], bf16)

            # ---------------- phase A ----------------
            with (
                tc.tile_pool(name="sload", bufs=2) as sload,
                tc.tile_pool(name="bload", bufs=1) as bload,
                tc.tile_pool(name="tpool", bufs=2) as tpool,
                tc.tile_pool(name="mpool", bufs=2) as mpool,
                tc.tile_pool(name="epool", bufs=2) as epool,
                tc.tile_pool(name="gevac", bufs=2) as gevac,
                tc.tile_pool(name="gpsum", bufs=2, space="PSUM") as gpsum,
            ):
                # batch: delta + a partials (t-major, contiguous reductions)
                bct = bload.tile([B, S, QS], bf16, tag="bc")
                nc.sync.dma_start(out=bct[:], in_=dbc[:, :, :])
                bnt = bload.tile([B, S, QS], bf16, tag="bn")
                nc.sync.dma_start(out=bnt[:], in_=dbn[:, :, :])
                nc.vector.tensor_add(delta[:], bct[:], bnt[:])
                dif = bload.tile([B, S, QS], bf16, tag="dif")
                nc.vector.tensor_sub(dif[:], bct[:], bnt[:])
                nc.vector.reduce_sum(out=cesb[:, 128:128 + S], in_=dif[:],
                                     axis=X)

                # embed partials
                ecols = persist.tile([B, 2], f32)
                for k, (ea, eb) in enumerate(((ehs, eht), (eds, edt))):
                    e1 = epool.tile([B, EPAD * H], bf16, tag="ea")
                    nc.scalar.dma_start(out=e1[:],
                                        in_=ea[:].rearrange("b t h -> b (t h)"))
                    e2 = epool.tile([B, EPAD * H], bf16, tag="eb")
                    nc.scalar.dma_start(out=e2[:],
                                        in_=eb[:].rearrange("b t h -> b (t h)"))
                    ed = epool.tile([B, EPAD * H], bf16, tag="ed")
                    nc.vector.tensor_sub(ed[:], e1[:], e2[:])
                    esq = epool.tile([B, EPAD * H], bf16, tag="esq")
                    nc.scalar.activation(esq[:], ed[:], Act.Square,
                                         accum_out=ecols[:, k:k + 1])
                embf = persist.tile([B, 1], f32)
                nc.vector.tensor_add(embf[:], ecols[:, 0:1], ecols[:, 1:2])
                nc.vector.tensor_copy(cesb[:, 192:193], embf[:])

                # grams + CE gathers per pair
                for p in range(3):
                    tp = tpool.tile([128, 2, T, 128], bf16, tag="tp")
                    nc.sync.dma_start(
                        out=tp[:, 0, :, :],
                        in_=xs[p][:, :, :].rearrange("b q t -> b (q t)"),
                        transpose=True)
                    nc.scalar.dma_start(
                        out=tp[:, 1, :, :],
                        in_=ys[p][:, :, :].rearrange("b q t -> b (q t)"),
                        transpose=True)

                    gpa = gpsum.tile([128, 256], f32, tag="ga")
                    gyy = gpsum.tile([128, 128], f32, tag="gy")
                    for t in range(T):
                        nc.tensor.matmul(
                            gpa[:], tp[:, 0, t, :], tp[:, :, t, :],
                            start=(t == 0), stop=(t == T - 1))
                        nc.tensor.matmul(
                            gyy[:], tp[:, 1, t, :], tp[:, 1, t, :],
                            start=(t == 0), stop=(t == T - 1))

                    # CE gather for this (student) pair (t-major, contiguous)
                    xtm = sload.tile([B, S, QS], bf16, tag="xs")
                    nc.sync.dma_start(out=xtm[:], in_=xst[p][:, :, :])
                    ms = mpool.tile([B, S, QS], bf16, tag="ms")
                    nc.vector.tensor_mul(ms[:], xtm[:], delta[:])
                    pcdst = (cesb[:, 64 * p:64 * p + S] if p < 2
                             else cesb2[:, 0:S])
                    nc.vector.reduce_sum(out=pcdst, in_=ms[:], axis=X)

                    # evacuate grams (bf16) + diag partials
                    gsb = gevac.tile([B, 384], bf16, tag="gsb")
                    nc.scalar.copy(gsb[:, 0:256], gpa[:])
                    nc.vector.tensor_copy(gsb[:, 256:384], gyy[:])
                    dsc = mpool.tile([B, 128], bf16, tag="dsc")
                    ddst = (cesb[:, 193 + 2 * p:195 + 2 * p] if p < 2
                            else cesb2[:, 64:66])
                    nc.vector.tensor_mul(dsc[:], gsb[:, 0:128], identb[:])
                    nc.vector.reduce_sum(out=ddst[:, 0:1], in_=dsc[:], axis=X)
                    dsc2 = mpool.tile([B, 128], bf16, tag="dsc")
                    nc.vector.tensor_mul(dsc2[:], gsb[:, 256:384], identb[:])
                    nc.vector.reduce_sum(out=ddst[:, 1:2], in_=dsc2[:], axis=X)

                    # stage mat blocks to collective input buffers
                    if p < 2:
                        xyr, xxr, yyr = 128 * p, 128 * (2 + p), 128 * (4 + p)
                        nc.sync.dma_start(out=c1_in[xyr:xyr + 128, :],
                                          in_=gsb[:, 128:256])
                        nc.sync.dma_start(out=c1_in[xxr:xxr + 128, :],
                                          in_=gsb[:, 0:128])
                        nc.sync.dma_start(out=c1_in[yyr:yyr + 128, :],
                                          in_=gsb[:, 256:384])
                    else:
                        nc.sync.dma_start(out=c2_in[0:128, :],
                                          in_=gsb[:, 128:256])
                        nc.sync.dma_start(out=c2_in[128:256, :],
                                          in_=gsb[:, 0:128])
                        nc.sync.dma_start(out=c2_in[256:384, :],
                                          in_=gsb[:, 256:384])

                    if p == 1:
                        # CE-extra block rides C1; fire C1 while pair 2 runs
                        nc.sync.dma_start(out=c1_in[768:896, :],
                                          in_=cesb[:, 0:128])
                        nc.sync.dma_start(out=c1_in[896:1024, :],
                                          in_=cesb[:, 128:256])
                        nc.gpsimd.collective_compute(
                            "AllReduce", Alu.add, replica_groups=groups,
                            ins=[c1_in[:, :]], outs=[c1_out[:, :]])
                nc.sync.dma_start(out=c2_in[384:512, :], in_=cesb2[:])
                nc.gpsimd.collective_compute(
                    "AllReduce", Alu.add, replica_groups=groups,
                    ins=[c2_in[:, :]], outs=[c2_out[:, :]])

            # ---------------- phase B ----------------
            with (
                tc.tile_pool(name="pbig", bufs=2) as pbig,
                tc.tile_pool(name="psmall", bufs=2) as psmall,
                tc.tile_pool(name="pconst", bufs=1) as pconst,
                tc.tile_pool(name="hps", bufs=2, space="PSUM") as hpsum,
                tc.tile_pool(name="fps", bufs=2, space="PSUM") as fpsum,
                tc.tile_pool(name="sps", bufs=1, space="PSUM") as spsum,
            ):
                # aux inputs
                axf = pconst.tile([B, 14], f32)
                nc.scalar.dma_start(out=axf[:], in_=auxf[:, :])
                axb = pconst.tile([B, 28], bf16)
                nc.scalar.dma_start(out=axb[:], in_=auxb[:, :])
                axi = pconst.tile([B, 2], i32)
                nc.scalar.dma_start(out=axi[:], in_=auxi[:, :])
                mskt = pconst.tile([2, 256], bf16)
                nc.scalar.dma_start(out=mskt[:], in_=msk_dram[:, :])
                ones2b = pconst.tile([2, 128], bf16)
                nc.vector.memset(ones2b[:], 1.0)
                ones_col = pconst.tile([B, 1], f32)
                nc.vector.memset(ones_col[:], 1.0)

                # CE-extra replicated blocks
                c1post = pconst.tile([B, 256], bf16)
                nc.sync.dma_start(out=c1post[:, 0:128], in_=c1_out[768:896, :])
                nc.sync.dma_start(out=c1post[:, 128:256], in_=c1_out[896:1024, :])
                c2post = pconst.tile([B, 128], bf16)
                nc.sync.dma_start(out=c2post[:], in_=c2_out[384:512, :])

                # slot gathers (per-core indices)
                r0 = pconst.tile([B, 128], bf16)
                nc.gpsimd.indirect_dma_start(
                    out=r0[:], out_offset=None,
                    in_=c1_out[0:768, :],
                    in_offset=bass.IndirectOffsetOnAxis(ap=axi[:, 0:1], axis=0))
                r1 = pconst.tile([B, 128], bf16)
                nc.gpsimd.indirect_dma_start(
                    out=r1[:], out_offset=None,
                    in_=c2_out[0:384, :],
                    in_offset=bass.IndirectOffsetOnAxis(ap=axi[:, 1:2], axis=0))
                t0p = fpsum.tile([128, 128], bf16, tag="t0")
                nc.tensor.transpose(t0p[:], r0[:], identb[:])
                t0 = pconst.tile([B, 128], bf16)
                nc.vector.tensor_copy(t0[:], t0p[:])
                t1p = fpsum.tile([128, 128], bf16, tag="t0")
                nc.tensor.transpose(t1p[:], r1[:], identb[:])
                t1 = pconst.tile([B, 128], bf16)
                nc.vector.tensor_copy(t1[:], t1p[:])
                # Gsl[:,0,:] = a0*r0 + (1-a0)*t1 ; Gsl[:,1,:] = a1*r1 + (1-a1)*t0
                Gsl = pconst.tile([B, 2, 128], bf16)
                gtmp = psmall.tile([B, 128], bf16, tag="gtmp")
                nc.vector.tensor_scalar(gtmp[:], t1[:], axf[:, 11:12], None,
                                        Alu.mult)
                nc.vector.scalar_tensor_tensor(Gsl[:, 0, :], r0[:], axf[:, 10:11],
                                               gtmp[:], Alu.mult, Alu.add)
                gtmp2 = psmall.tile([B, 128], bf16, tag="gtmp")
                nc.vector.tensor_scalar(gtmp2[:], t0[:], axf[:, 13:14], None,
                                        Alu.mult)
                nc.vector.scalar_tensor_tensor(Gsl[:, 1, :], r1[:], axf[:, 12:13],
                                               gtmp2[:], Alu.mult, Alu.add)

                # diag6 + per-slot D2 / DH via shipped selection masks
                diag6 = pconst.tile([B, 6], bf16)
                nc.vector.tensor_copy(diag6[:, 0:4], c1post[:, 193:197])
                nc.vector.tensor_copy(diag6[:, 4:6], c2post[:, 64:66])
                D2 = pconst.tile([B, 2], f32)
                DH = pconst.tile([B, 2], f32)
                for s in range(2):
                    selr = axb[:, 4 + 6 * s:10 + 6 * s]
                    selc = axb[:, 16 + 6 * s:22 + 6 * s]
                    dt_ = psmall.tile([B, 6], bf16, tag="dt")
                    nc.vector.tensor_mul(dt_[:], diag6[:], selr)
                    nc.vector.reduce_sum(out=D2[:, s:s + 1], in_=dt_[:], axis=X)
                    dt2 = psmall.tile([B, 6], bf16, tag="dt")
                    nc.vector.tensor_mul(dt2[:], diag6[:], selc)
                    nc.vector.reduce_sum(out=DH[:, s:s + 1], in_=dt2[:], axis=X)

                F = pconst.tile([B, 2], f32)
                nc.vector.memset(F[:], 0.0)
                sv = pconst.tile([B, 2], f32)
                mv = pconst.tile([B, 2], f32)
                scrx = pconst.tile([B, 2, 128], bf16)
                escr = pconst.tile([B, 128], bf16)

                # ln on DVE: exponent/mantissa split + deg-5 poly (keeps the
                # scalar engine Exp-only -> no act-table reloads)
                LN2 = 0.6931471805599453
                PA = (0.99988786, -0.49636758, 0.30467027, -0.15602615,
                      0.04106372)

                def dve_ln(dst, src, n):
                    svi = src.bitcast(i32)
                    sh = psmall.tile([B, n], i32, tag="lsh")
                    nc.vector.tensor_scalar(sh[:], svi, 23, None,
                                            Alu.logical_shift_right)
                    ef = psmall.tile([B, n], f32, tag="lef")
                    nc.vector.tensor_copy(ef[:], sh[:])
                    mi = psmall.tile([B, n], i32, tag="lmi")
                    nc.vector.tensor_scalar(mi[:], svi, 0x007FFFFF, 0x3F800000,
                                            Alu.bitwise_and, Alu.bitwise_or)
                    tt_ = psmall.tile([B, n], f32, tag="ltt")
                    nc.vector.tensor_scalar(tt_[:], mi[:].bitcast(f32), 1.0,
                                            None, Alu.subtract)
                    hp = psmall.tile([B, n], f32, tag="lhp")
                    nc.vector.tensor_scalar(hp[:], tt_[:], PA[4], PA[3],
                                            Alu.mult, Alu.add)
                    for ak in (PA[2], PA[1], PA[0]):
                        hm = psmall.tile([B, n], f32, tag="lhm")
                        nc.vector.tensor_tensor(hm[:], hp[:], tt_[:], Alu.mult)
                        hp = psmall.tile([B, n], f32, tag="lhp")
                        nc.vector.tensor_scalar(hp[:], hm[:], ak, None, Alu.add)
                    pv = psmall.tile([B, n], f32, tag="lpv")
                    nc.vector.tensor_tensor(pv[:], hp[:], tt_[:], Alu.mult)
                    e2f = psmall.tile([B, n], f32, tag="le2")
                    nc.vector.tensor_scalar(e2f[:], ef[:], LN2, -127.0 * LN2,
                                            Alu.mult, Alu.add)
                    nc.vector.tensor_tensor(dst, e2f[:], pv[:], Alu.add)

                for eps in _eps_schedule():
                    damp = 1.0 / (1.0 + eps / RHO)
                    c = GSCALE / eps
                    fsum = psmall.tile([B, 2], f32, tag="fsum")
                    nc.vector.tensor_add(fsum[:], F[:], DH[:])
                    ftp = fpsum.tile([2, 128], f32, tag="ft")
                    nc.tensor.transpose(ftp[:], fsum[:], identf[:])
                    HT = psmall.tile([2, 128], bf16, tag="ht")
                    nc.vector.tensor_scalar(HT[:], ftp[:], 1.0 / GSCALE,
                                            blog * eps / GSCALE,
                                            Alu.mult, Alu.add)
                    rhm = psmall.tile([2, 2, 128], bf16, tag="rhm")
                    nc.vector.tensor_tensor(
                        rhm[:], HT[:].unsqueeze(1).broadcast_to((2, 2, 128)),
                        mskt[:].rearrange("k (a j) -> k a j", j=128), Alu.mult)
                    hbt = hpsum.tile([128, 256], f32, tag="hb")
                    nc.tensor.matmul(hbt[:], ones2b[:],
                                     rhm[:].rearrange("k a j -> k (a j)"),
                                     start=True, stop=False)
                    nc.tensor.matmul(hbt[:], identb[:],
                                     Gsl[:].rearrange("b a j -> b (a j)"),
                                     start=False, stop=True)
                    hb3 = hbt[:].rearrange("b (s j) -> b s j", j=128)
                    nc.vector.reduce_max(out=mv[:], in_=hb3, axis=X)
                    nc.vector.tensor_tensor(
                        scrx[:], hb3,
                        mv[:].unsqueeze(2).broadcast_to((B, 2, 128)),
                        Alu.subtract)
                    for s in range(2):
                        nc.scalar.activation(escr[:], scrx[:, s, :], Act.Exp,
                                             scale=float(c),
                                             accum_out=sv[:, s:s + 1])
                    lg = psmall.tile([B, 2], f32, tag="lg")
                    dve_ln(lg[:], sv[:], 2)
                    # dmu = D2 - eps*lg - GSCALE*mv
                    dm1 = psmall.tile([B, 2], f32, tag="dm1")
                    nc.vector.scalar_tensor_tensor(dm1[:], lg[:], float(-eps),
                                                   D2[:], Alu.mult, Alu.add)
                    dmu = psmall.tile([B, 2], f32, tag="dmu")
                    nc.vector.scalar_tensor_tensor(dmu[:], mv[:], float(-GSCALE),
                                                   dm1[:], Alu.mult, Alu.add)
                    dr = psmall.tile([B, 2], f32, tag="dr")
                    nc.vector.tensor_copy(dr[:, 0:1], dmu[:, 1:2])
                    nc.vector.tensor_copy(dr[:, 1:2], dmu[:, 0:1])
                    # cmix = damp * ((1-pf)*dmu + pf*rev(dmu))
                    c1t = psmall.tile([B, 2], f32, tag="c1t")
                    nc.vector.tensor_scalar(c1t[:], dmu[:], axf[:, 1:2],
                                            float(damp), Alu.mult, Alu.mult)
                    c2t = psmall.tile([B, 2], f32, tag="c2t")
                    nc.vector.tensor_scalar(c2t[:], dr[:], axf[:, 0:1],
                                            float(damp), Alu.mult, Alu.mult)
                    cmix = psmall.tile([B, 2], f32, tag="cmix")
                    nc.vector.tensor_add(cmix[:], c1t[:], c2t[:])
                    # F = wF*F + vF*cmix
                    m1 = psmall.tile([B, 2], f32, tag="m1")
                    nc.vector.tensor_mul(m1[:], F[:], axf[:, 2:4])
                    m2 = psmall.tile([B, 2], f32, tag="m2")
                    nc.vector.tensor_mul(m2[:], cmix[:], axf[:, 4:6])
                    nc.vector.tensor_add(F[:], m1[:], m2[:])

                # ---- loss_kd partial ----
                E2 = psmall.tile([B, 2], f32, tag="e2")
                nc.scalar.activation(E2[:], F[:], Act.Exp, scale=float(-1.0 / RHO))
                km = psmall.tile([B, 2], f32, tag="km")
                nc.vector.tensor_mul(km[:], E2[:], axf[:, 6:8])
                kdp = psmall.tile([B, 1], f32, tag="kdp")
                nc.vector.reduce_sum(out=kdp[:], in_=km[:], axis=X)

                # ---- CE (replicated; gated by aux csup/cemb) ----
                pcall = pconst.tile([B, 192], f32)
                nc.vector.tensor_copy(pcall[:, 0:128], c1post[:, 0:128])
                nc.vector.tensor_copy(pcall[:, 128:192], c2post[:, 0:64])
                af = pconst.tile([B, 64], f32)
                nc.vector.tensor_copy(af[:], c1post[:, 128:192])
                embcol = pconst.tile([B, 1], f32)
                nc.vector.tensor_copy(embcol[:], c1post[:, 192:193])

                idxf = pconst.tile([B, 64], f32)
                nc.scalar.dma_start(out=idxf[:], in_=idx_dram[:, :])
                pos = psmall.tile([B, 64], f32, tag="pos")
                nc.vector.tensor_scalar(pos[:], pcall[:, 0:64], 0.0, None,
                                        Alu.is_gt)
                ip1 = psmall.tile([B, 64], f32, tag="ip1")
                nc.vector.scalar_tensor_tensor(ip1[:], idxf[:], 1.0, pos[:],
                                               Alu.add, Alu.mult)
                Lp = psmall.tile([B, 1], f32, tag="Lp")
                nc.vector.reduce_max(out=Lp[:], in_=ip1[:], axis=X)
                eq0 = psmall.tile([B, 1], f32, tag="eq0")
                nc.vector.tensor_scalar(eq0[:], Lp[:], 0.0, None, Alu.is_equal)
                Lv = psmall.tile([B, 1], f32, tag="Lv")
                nc.vector.scalar_tensor_tensor(Lv[:], eq0[:], float(S), Lp[:],
                                               Alu.mult, Alu.add)
                dl = psmall.tile([B, 64], f32, tag="dl")
                nc.vector.tensor_scalar(dl[:], idxf[:], Lv[:, 0:1], None,
                                        Alu.subtract)
                mask = psmall.tile([B, 64], f32, tag="mask")
                nc.vector.tensor_scalar(mask[:], dl[:], 0.0, None, Alu.is_lt)
                negf = psmall.tile([B, 64], f32, tag="negf")
                nc.vector.tensor_scalar(negf[:], mask[:], 1.0, 1e9,
                                        Alu.subtract, Alu.mult)
                # a = floor((asum+1)/2) via magic round (values < 2^22)
                MAGIC = 12582912.0
                tv = psmall.tile([B, 64], f32, tag="tv")
                nc.vector.tensor_scalar(tv[:], af[:], 0.5, 1024.25,
                                        Alu.mult, Alu.add)
                tm = psmall.tile([B, 64], f32, tag="tm")
                nc.vector.tensor_scalar(tm[:], tv[:], MAGIC, MAGIC,
                                        Alu.add, Alu.subtract)
                av = psmall.tile([B, 64], f32, tag="av")
                nc.vector.tensor_scalar(av[:], tm[:], 1024.0, None, Alu.subtract)
                amask = psmall.tile([B, 64], f32, tag="amask")
                nc.vector.tensor_tensor(amask[:], av[:], mask[:], Alu.mult)
                pc3 = pcall[:].rearrange("b (s q) -> b s q", q=64)
                mce = pbig.tile([B, 3, 64], f32, tag="mce")
                mask3 = mask[:].unsqueeze(1).broadcast_to((B, 3, 64))
                negf3 = negf[:].unsqueeze(1).broadcast_to((B, 3, 64))
                amask3 = amask[:].unsqueeze(1).broadcast_to((B, 3, 64))
                t2_ = pbig.tile([B, 3, 64], f32, tag="tt")
                nc.vector.scalar_tensor_tensor(t2_[:], pc3, 2.0, mask3, Alu.mult,
                                               Alu.mult)
                nc.vector.tensor_tensor(mce[:], t2_[:], negf3, Alu.add)
                mx3 = psmall.tile([B, 3], f32, tag="mx3")
                nc.vector.reduce_max(out=mx3[:], in_=mce[:], axis=X)
                mb3 = mx3[:].unsqueeze(2).broadcast_to((B, 3, 64))
                dd = pbig.tile([B, 3, 64], f32, tag="dd")
                nc.vector.tensor_tensor(dd[:], mce[:], mb3, Alu.subtract)
                ee = pbig.tile([B, 3, 64], f32, tag="ee")
                nc.scalar.activation(ee[:], dd[:], Act.Exp)
                ss3 = psmall.tile([B, 3], f32, tag="ss3")
                nc.vector.reduce_sum(out=ss3[:], in_=ee[:], axis=X)
                lg3 = psmall.tile([B, 3], f32, tag="lg3")
                dve_ln(lg3[:], ss3[:], 3)
                lse3 = psmall.tile([B, 3], f32, tag="lse3")
                nc.vector.tensor_add(lse3[:], mx3[:], lg3[:])
                lb3 = lse3[:].unsqueeze(2).broadcast_to((B, 3, 64))
                d1 = pbig.tile([B, 3, 64], f32, tag="dd")
                nc.vector.tensor_tensor(d1[:], mce[:], lb3, Alu.subtract)
                d2_ = pbig.tile([B, 3, 64], f32, tag="tt")
                nc.vector.tensor_tensor(d2_[:], d1[:], amask3, Alu.mult)
                rowsum = psmall.tile([B, 1], f32, tag="rs")
                nc.vector.reduce_sum(out=rowsum[:],
                                     in_=d2_[:].rearrange("b s q -> b (s q)"),
                                     axis=X)

                # ---- final combine: csup*CE + cemb*embed + kd_partial ----
                tot_ps = spsum.tile([1, 1], f32, tag="tot")
                nc.tensor.matmul(tot_ps[:], rowsum[:], axf[:, 8:9], start=True,
                                 stop=False)
                nc.tensor.matmul(tot_ps[:], embcol[:], axf[:, 9:10], start=False,
                                 stop=False)
                nc.tensor.matmul(tot_ps[:], kdp[:], ones_col[:], start=False,
                                 stop=True)
                outt = psmall.tile([1, 1], f32, tag="outt")
                nc.vector.tensor_copy(outt[:], tot_ps[:])
                nc.sync.dma_start(out=out_ext[:, :], in_=outt[:])

    nc.compile()
    return nc


_NC = None
LAST_RESULTS = None


def _core_aux(c):
    sl = SLOTS[c]
    i0 = C1MAT[sl['i0']] if sl['i0'] is not None else 0
    i1 = C2MAT[sl['i1']] if sl['i1'] is not None else 0
    pf = float(sl['pf'])
    wf = [0.0, 0.0] if sl['pf'] else [0.5, 0.5]
    vf = [1.0, 1.0] if sl['pf'] else [0.5, 0.5]
    cgate = 1.0 if c == 0 else 0.0
    auxf = np.zeros((B, 14), np.float32)
    auxf[:, 0] = pf
    auxf[:, 1] = 1.0 - pf
    auxf[:, 2:4] = wf
    auxf[:, 4:6] = vf
    auxf[:, 6:8] = sl['kc']
    auxf[:, 8] = -LOSS_WEIGHT * SUP_W * cgate
    auxf[:, 9] = LOSS_WEIGHT * EMBED_W * 0.5 * cgate
    auxf[:, 10] = float(sl['a0'])
    auxf[:, 11] = 1.0 - float(sl['a0'])
    auxf[:, 12] = float(sl['a1'])
    auxf[:, 13] = 1.0 - float(sl['a1'])
    auxb = np.zeros((B, 28), np.float32)
    auxb[:, 0] = float(sl['a0'])
    auxb[:, 1] = 1.0 - float(sl['a0'])
    auxb[:, 2] = float(sl['a1'])
    auxb[:, 3] = 1.0 - float(sl['a1'])
    for s in range(2):
        auxb[:, 4 + 6 * s + DIDX[sl['rs'][s]]] = 2.0
        auxb[:, 16 + 6 * s + DIDX[sl['cs'][s]]] = -2.0
    auxi = np.zeros((B, 2), np.int32)
    auxi[:, 0] = 128 * i0 + np.arange(B)
    auxi[:, 1] = 128 * i1 + np.arange(B)
    return auxf, auxb, auxi


def _shard_inputs(logit_c, logit_t, logit_ensemble, logit_teacher_c,
                  logit_teacher_t, logit_teacher_ensemble, out_h_student,
                  out_h_teacher, out_d_student, out_d_teacher, batch):
    import ml_dtypes
    bf = ml_dtypes.bfloat16
    students = [logit_c, logit_t, logit_ensemble]
    teachers = [logit_teacher_c, logit_teacher_t, logit_teacher_ensemble]
    embeds = dict(ehs=out_h_student, eht=out_h_teacher,
                  eds=out_d_student, edt=out_d_teacher)
    # q-major [B, QS, T] bf16 per core (XBAR transpose source) + t-major
    # [B, S, QS] copies for the contiguous CE/delta path
    sbf = [np.asarray(a, np.float32).astype(bf) for a in students]
    sb = [np.ascontiguousarray(np.transpose(a, (0, 2, 1))) for a in sbf]
    tb = [np.ascontiguousarray(np.transpose(
        np.asarray(a, np.float32).astype(bf), (0, 2, 1))) for a in teachers]
    bct = np.asarray(batch[:, 1:1 + S, :Q], np.float32).astype(bf)
    bnt = np.asarray(batch[:, 1:1 + S, Q:], np.float32).astype(bf)
    in_maps = []
    for c in range(NCORES):
        q0 = QS * c
        m = {}
        for nm, arr in zip(("xc", "xt", "xe"), sb):
            m[nm] = np.ascontiguousarray(arr[:, q0:q0 + QS, :])
        for nm, arr in zip(("yc", "yt", "ye"), tb):
            m[nm] = np.ascontiguousarray(arr[:, q0:q0 + QS, :])
        for nm, arr in zip(("xct", "xtt", "xet"), sbf):
            m[nm] = np.ascontiguousarray(arr[:, 0:S, q0:q0 + QS])
        m["dbc"] = np.ascontiguousarray(bct[:, :, q0:q0 + QS])
        m["dbn"] = np.ascontiguousarray(bnt[:, :, q0:q0 + QS])
        t0, w = EOFF[c], ESPLIT[c]
        for nm, arr in embeds.items():
            sl = np.zeros((B, EPAD, H), bf)
            sl[:, :w, :] = np.asarray(arr[:, t0:t0 + w, :], np.float32).astype(bf)
            m[nm] = sl
        axf, axb, axi = _core_aux(c)
        m["auxf"] = axf
        m["auxb"] = axb.astype(bf)
        m["auxi"] = axi
        in_maps.append(m)
    return in_maps


def kernel(**inputs):
    global _NC, LAST_RESULTS
    from concourse.bass_utils import run_bass_kernel_spmd
    if _NC is None:
        _NC = build_bass()
    in_maps = _shard_inputs(**inputs)
    trace = bool(int(os.environ.get("KERNEL_TRACE", "0")))
    res = run_bass_kernel_spmd(_NC, in_maps, list(range(NCORES)), trace=trace)
    LAST_RESULTS = res
    total = sum(float(np.asarray(r["out"]).reshape(-1)[0]) for r in res.results)
    return np.asarray([total], dtype=np.float32)


# revision 20
# speedup vs baseline: 1.6629x; 1.1363x over previous
"""Trainium2 Bass kernel for nn_CombinedLoss (sinkhorn-KD + soft-CE + embed MSE).

v2 architecture (8 cores):
  - All inputs shipped bf16. Logits in q-major [B, QS, T] per-core layout so a
    single XBAR DMA-transpose per tensor yields [q, t, b] gram operands
    (no PE transposes, no PSUM evacuation of transposed data).
  - Grams per pair: [G_xx | G_xy] and G_yy only; G_yx derived on the owner
    core as G_xy^T after reduction.
  - Two bf16 AllReduces in [mat*128+b, j] block layout: C1 (pairs 0/1 grams +
    CE/a/embed/diag cols) fires after pair 1 and hides under pair-2 compute;
    C2 (pair-2 grams + pc2/diag2) is the only exposed collective.
  - Phase B (9 sinkhorn iterations) sharded: each core processes only 2 of
    the 12 B x B matrices, selected via per-core indirect-DMA row gathers and
    data-driven update rules (uniform SPMD program).
  - Final loss_kd partials stay per-core; the host sums the 8 scalar outputs
    (the unshard step - all loss reductions are sums over shards).
"""
import os
import numpy as np

B = 128
T = 50
Q = 1024
S = 49          # MAX_STEP - 1
H = 256
NCORES = 8
QS = Q // NCORES
TEMP = 0.5
GSCALE = 1.0 / (TEMP * TEMP)   # p-gram = GSCALE * logit-gram = 4
RHO = 500.0 ** 2
EPS_FINAL = 0.005 ** 2
SUP_W, DIST_W, EMBED_W, LOSS_WEIGHT = 1.0, 0.01, 1.0, 1.0
CKD = float(LOSS_WEIGHT * DIST_W * (RHO + EPS_FINAL / 2.0) / B)

# embed t-shard split (padded to 7 per core)
ESPLIT = [7, 7, 6, 6, 6, 6, 6, 6]
EOFF = [0, 7, 14, 20, 26, 32, 38, 44]
EPAD = 7

# c1 AllReduce buffer: [1024, 128] bf16 rows; mat k occupies rows 128k..128k+127
# mats: xy0=0, xy1=1, xx0=2, xx1=3, yy0=4, yy1=5; CE-extra blocks rows 768-1023
# CE-extra cols (within the logical [B, 256] block):
#   pc0 at 0-63, pc1 64-127, a 128-191, embed 192, diag01 193-196
C1ROWS = 1024
# c2: xy2=0, xx2=1, yy2=2; CE2-extra rows 384-511: pc2 0-63, diag2 64-65
C2ROWS = 512

C1MAT = {('xy', 0): 0, ('xy', 1): 1, ('xx', 0): 2, ('xx', 1): 3,
         ('yy', 0): 4, ('yy', 1): 5}
C2MAT = {('xy', 2): 0, ('xx', 2): 1, ('yy', 2): 2}

# per-core slot assignment: (slot0 src, slot1 src); None = derived/dead
# a0=1: slot0 <- c1 read; a0=0: slot0 <- transpose(c2 read). etc.
SLOTS = {
    0: dict(i0=('xy', 0), i1=None, a0=1, a1=0, pf=1,
            kc=[-CKD, -CKD], rs=[('x', 0), ('y', 0)], cs=[('y', 0), ('x', 0)]),
    1: dict(i0=('xy', 1), i1=None, a0=1, a1=0, pf=1,
            kc=[-CKD, -CKD], rs=[('x', 1), ('y', 1)], cs=[('y', 1), ('x', 1)]),
    2: dict(i0=None, i1=('xy', 2), a0=0, a1=1, pf=1,
            kc=[-CKD, -CKD], rs=[('y', 2), ('x', 2)], cs=[('x', 2), ('y', 2)]),
    3: dict(i0=('xx', 0), i1=('xy', 2), a0=1, a1=1, pf=0,
            kc=[CKD, 0.0], rs=[('x', 0), ('x', 2)], cs=[('x', 0), ('y', 2)]),
    4: dict(i0=('xx', 1), i1=('xy', 2), a0=1, a1=1, pf=0,
            kc=[CKD, 0.0], rs=[('x', 1), ('x', 2)], cs=[('x', 1), ('y', 2)]),
    5: dict(i0=('xy', 0), i1=('xx', 2), a0=1, a1=1, pf=0,
            kc=[0.0, CKD], rs=[('x', 0), ('x', 2)], cs=[('y', 0), ('x', 2)]),
    6: dict(i0=('yy', 0), i1=('yy', 2), a0=1, a1=1, pf=0,
            kc=[CKD, CKD], rs=[('y', 0), ('y', 2)], cs=[('y', 0), ('y', 2)]),
    7: dict(i0=('yy', 1), i1=('xy', 2), a0=1, a1=1, pf=0,
            kc=[CKD, 0.0], rs=[('y', 1), ('y', 2)], cs=[('y', 1), ('y', 2)]),
}
# diag6 order: [dxx0, dyy0, dxx1, dyy1, dxx2, dyy2]
DIDX = {('x', 0): 0, ('y', 0): 1, ('x', 1): 2, ('y', 1): 3,
        ('x', 2): 4, ('y', 2): 5}


def _eps_schedule():
    eps_list = []
    e = 1.0
    while e > EPS_FINAL:
        eps_list.append(e)
        e = e * 0.25
    eps_list.append(EPS_FINAL)
    return eps_list


def build_bass():
    import concourse.bass as bass
    import concourse.bacc as bacc
    import concourse.tile as tile
    from concourse import mybir
    from concourse.masks import make_identity

    f32 = mybir.dt.float32
    bf16 = mybir.dt.bfloat16
    i32 = mybir.dt.int32
    Alu = mybir.AluOpType
    Act = mybir.ActivationFunctionType
    X = mybir.AxisListType.X

    nc = bacc.Bacc(
        "TRN2",
        target_bir_lowering=False,
        debug=False,
        num_devices=NCORES,
    )

    xs = [nc.declare_dram_parameter(n, [QS, T, B], bf16, isOutput=False)
          for n in ("xc", "xt", "xe")]
    ys = [nc.declare_dram_parameter(n, [QS, T, B], bf16, isOutput=False)
          for n in ("yc", "yt", "ye")]
    xst = [nc.declare_dram_parameter(n, [B, S, QS], bf16, isOutput=False)
           for n in ("xct", "xtt", "xet")]
    dbc = nc.declare_dram_parameter("dbc", [B, S, QS], bf16, isOutput=False)
    dbn = nc.declare_dram_parameter("dbn", [B, S, QS], bf16, isOutput=False)
    ehs = nc.declare_dram_parameter("ehs", [B, EPAD, H], bf16, isOutput=False)
    eht = nc.declare_dram_parameter("eht", [B, EPAD, H], bf16, isOutput=False)
    eds = nc.declare_dram_parameter("eds", [B, EPAD, H], bf16, isOutput=False)
    edt = nc.declare_dram_parameter("edt", [B, EPAD, H], bf16, isOutput=False)
    auxf = nc.declare_dram_parameter("auxf", [B, 14], f32, isOutput=False)
    auxb = nc.declare_dram_parameter("auxb", [B, 28], bf16, isOutput=False)
    auxi = nc.declare_dram_parameter("auxi", [B, 2], i32, isOutput=False)
    out_ext = nc.declare_dram_parameter("out", [1, 1], f32, isOutput=True)

    c1_in = nc.dram_tensor("c1_in", [C1ROWS, 128], bf16)
    c1_out = nc.dram_tensor("c1_out", [C1ROWS, 128], bf16, addr_space="Shared")
    c2_in = nc.dram_tensor("c2_in", [C2ROWS, 128], bf16)
    c2_out = nc.dram_tensor("c2_out", [C2ROWS, 128], bf16, addr_space="Shared")

    # constants baked into the NEFF
    import ml_dtypes
    msk_np = np.zeros((2, 256), np.float32)
    msk_np[0, 0:128] = 1.0
    msk_np[1, 128:256] = 1.0
    msk_dram = nc.inline_tensor(msk_np.astype(ml_dtypes.bfloat16), "mskc")
    idx_np = np.broadcast_to(np.arange(64, dtype=np.float32), (B, 64)).copy()
    idx_dram = nc.inline_tensor(idx_np, "idxc")

    blog = float(-np.log(float(B)))
    groups = [list(range(NCORES))]

    with nc.allow_low_precision(reason="bf16 partial sums are exact or slack-validated"), \
         tile.TileContext(nc) as tc:
        with tc.tile_pool(name="persist", bufs=1) as persist:
            identf = persist.tile([128, 128], f32)
            make_identity(nc, identf[:])
            identb = persist.tile([128, 128], bf16)
            nc.vector.tensor_copy(identb[:], identf[:])

            cesb = persist.tile([B, 256], bf16)
            nc.vector.memset(cesb[:], 0.0)
            cesb2 = persist.tile([B, 128], bf16)
            nc.vector.memset(cesb2[:], 0.0)
            delta = persist.tile([B, S, QS], bf16)

            # ---------------- phase A ----------------
            with (
                tc.tile_pool(name="sload", bufs=2) as sload,
                tc.tile_pool(name="bload", bufs=1) as bload,
                tc.tile_pool(name="tpool", bufs=2) as tpool,
                tc.tile_pool(name="mpool", bufs=2) as mpool,
                tc.tile_pool(name="epool", bufs=2) as epool,
                tc.tile_pool(name="gevac", bufs=2) as gevac,
                tc.tile_pool(name="gpsum", bufs=2, space="PSUM") as gpsum,
            ):
                # batch: delta + a partials (t-major, contiguous reductions)
                bct = bload.tile([B, S, QS], bf16, tag="bc")
                nc.sync.dma_start(out=bct[:], in_=dbc[:, :, :])
                bnt = bload.tile([B, S, QS], bf16, tag="bn")
                nc.sync.dma_start(out=bnt[:], in_=dbn[:, :, :])
                nc.vector.tensor_add(delta[:], bct[:], bnt[:])
                dif = bload.tile([B, S, QS], bf16, tag="dif")
                nc.vector.tensor_sub(dif[:], bct[:], bnt[:])
                nc.vector.reduce_sum(out=cesb[:, 128:128 + S], in_=dif[:],
                                     axis=X)

                # embed partials
                ecols = persist.tile([B, 2], f32)
                for k, (ea, eb) in enumerate(((ehs, eht), (eds, edt))):
                    e1 = epool.tile([B, EPAD * H], bf16, tag="ea")
                    nc.scalar.dma_start(out=e1[:],
                                        in_=ea[:].rearrange("b t h -> b (t h)"))
                    e2 = epool.tile([B, EPAD * H], bf16, tag="eb")
                    nc.scalar.dma_start(out=e2[:],
                                        in_=eb[:].rearrange("b t h -> b (t h)"))
                    ed = epool.tile([B, EPAD * H], bf16, tag="ed")
                    nc.vector.tensor_sub(ed[:], e1[:], e2[:])
                    esq = epool.tile([B, EPAD * H], bf16, tag="esq")
                    nc.scalar.activation(esq[:], ed[:], Act.Square,
                                         accum_out=ecols[:, k:k + 1])
                embf = persist.tile([B, 1], f32)
                nc.vector.tensor_add(embf[:], ecols[:, 0:1], ecols[:, 1:2])
                nc.vector.tensor_copy(cesb[:, 192:193], embf[:])

                # grams + CE gathers per pair
                for p in range(3):
                    tp = tpool.tile([128, 2, T, 128], bf16, tag="tp")
                    nc.sync.dma_start(out=tp[:, 0, :, :], in_=xs[p][:, :, :])
                    nc.scalar.dma_start(out=tp[:, 1, :, :], in_=ys[p][:, :, :])

                    gpa = gpsum.tile([128, 256], f32, tag="ga")
                    gyy = gpsum.tile([128, 128], f32, tag="gy")
                    for t in range(T):
                        nc.tensor.matmul(
                            gpa[:], tp[:, 0, t, :], tp[:, :, t, :],
                            start=(t == 0), stop=(t == T - 1))
                        nc.tensor.matmul(
                            gyy[:], tp[:, 1, t, :], tp[:, 1, t, :],
                            start=(t == 0), stop=(t == T - 1))

                    # CE gather for this (student) pair (t-major, contiguous)
                    xtm = sload.tile([B, S, QS], bf16, tag="xs")
                    nc.sync.dma_start(out=xtm[:], in_=xst[p][:, :, :])
                    ms = mpool.tile([B, S, QS], bf16, tag="ms")
                    nc.vector.tensor_mul(ms[:], xtm[:], delta[:])
                    pcdst = (cesb[:, 64 * p:64 * p + S] if p < 2
                             else cesb2[:, 0:S])
                    nc.vector.reduce_sum(out=pcdst, in_=ms[:], axis=X)

                    # evacuate grams (bf16) + diag partials
                    gsb = gevac.tile([B, 384], bf16, tag="gsb")
                    nc.scalar.copy(gsb[:, 0:256], gpa[:])
                    nc.vector.tensor_copy(gsb[:, 256:384], gyy[:])
                    dsc = mpool.tile([B, 128], bf16, tag="dsc")
                    ddst = (cesb[:, 193 + 2 * p:195 + 2 * p] if p < 2
                            else cesb2[:, 64:66])
                    nc.vector.tensor_mul(dsc[:], gsb[:, 0:128], identb[:])
                    nc.vector.reduce_sum(out=ddst[:, 0:1], in_=dsc[:], axis=X)
                    dsc2 = mpool.tile([B, 128], bf16, tag="dsc")
                    nc.vector.tensor_mul(dsc2[:], gsb[:, 256:384], identb[:])
                    nc.vector.reduce_sum(out=ddst[:, 1:2], in_=dsc2[:], axis=X)

                    # stage mat blocks to collective input buffers
                    if p < 2:
                        xyr, xxr, yyr = 128 * p, 128 * (2 + p), 128 * (4 + p)
                        nc.sync.dma_start(out=c1_in[xyr:xyr + 128, :],
                                          in_=gsb[:, 128:256])
                        nc.sync.dma_start(out=c1_in[xxr:xxr + 128, :],
                                          in_=gsb[:, 0:128])
                        nc.sync.dma_start(out=c1_in[yyr:yyr + 128, :],
                                          in_=gsb[:, 256:384])
                    else:
                        nc.sync.dma_start(out=c2_in[0:128, :],
                                          in_=gsb[:, 128:256])
                        nc.sync.dma_start(out=c2_in[128:256, :],
                                          in_=gsb[:, 0:128])
                        nc.sync.dma_start(out=c2_in[256:384, :],
                                          in_=gsb[:, 256:384])

                    if p == 1:
                        # CE-extra block rides C1; fire C1 while pair 2 runs
                        nc.sync.dma_start(out=c1_in[768:896, :],
                                          in_=cesb[:, 0:128])
                        nc.sync.dma_start(out=c1_in[896:1024, :],
                                          in_=cesb[:, 128:256])
                        nc.gpsimd.collective_compute(
                            "AllReduce", Alu.add, replica_groups=groups,
                            ins=[c1_in[:, :]], outs=[c1_out[:, :]])
                nc.sync.dma_start(out=c2_in[384:512, :], in_=cesb2[:])
                nc.gpsimd.collective_compute(
                    "AllReduce", Alu.add, replica_groups=groups,
                    ins=[c2_in[:, :]], outs=[c2_out[:, :]])

            # ---------------- phase B ----------------
            with (
                tc.tile_pool(name="pbig", bufs=2) as pbig,
                tc.tile_pool(name="psmall", bufs=2) as psmall,
                tc.tile_pool(name="pconst", bufs=1) as pconst,
                tc.tile_pool(name="hps", bufs=2, space="PSUM") as hpsum,
                tc.tile_pool(name="fps", bufs=2, space="PSUM") as fpsum,
                tc.tile_pool(name="sps", bufs=1, space="PSUM") as spsum,
            ):
                # aux inputs
                axf = pconst.tile([B, 14], f32)
                nc.scalar.dma_start(out=axf[:], in_=auxf[:, :])
                axb = pconst.tile([B, 28], bf16)
                nc.scalar.dma_start(out=axb[:], in_=auxb[:, :])
                axi = pconst.tile([B, 2], i32)
                nc.scalar.dma_start(out=axi[:], in_=auxi[:, :])
                mskt = pconst.tile([2, 256], bf16)
                nc.scalar.dma_start(out=mskt[:], in_=msk_dram[:, :])
                ones2b = pconst.tile([2, 128], bf16)
                nc.vector.memset(ones2b[:], 1.0)
                ones_col = pconst.tile([B, 1], f32)
                nc.vector.memset(ones_col[:], 1.0)

                # CE-extra replicated blocks
                c1post = pconst.tile([B, 256], bf16)
                nc.sync.dma_start(out=c1post[:, 0:128], in_=c1_out[768:896, :])
                nc.sync.dma_start(out=c1post[:, 128:256], in_=c1_out[896:1024, :])
                c2post = pconst.tile([B, 128], bf16)
                nc.sync.dma_start(out=c2post[:], in_=c2_out[384:512, :])

                # slot gathers (per-core indices)
                r0 = pconst.tile([B, 128], bf16)
                nc.gpsimd.indirect_dma_start(
                    out=r0[:], out_offset=None,
                    in_=c1_out[0:768, :],
                    in_offset=bass.IndirectOffsetOnAxis(ap=axi[:, 0:1], axis=0))
                r1 = pconst.tile([B, 128], bf16)
                nc.gpsimd.indirect_dma_start(
                    out=r1[:], out_offset=None,
                    in_=c2_out[0:384, :],
                    in_offset=bass.IndirectOffsetOnAxis(ap=axi[:, 1:2], axis=0))
                t0p = fpsum.tile([128, 128], bf16, tag="t0")
                nc.tensor.transpose(t0p[:], r0[:], identb[:])
                t0 = pconst.tile([B, 128], bf16)
                nc.vector.tensor_copy(t0[:], t0p[:])
                t1p = fpsum.tile([128, 128], bf16, tag="t0")
                nc.tensor.transpose(t1p[:], r1[:], identb[:])
                t1 = pconst.tile([B, 128], bf16)
                nc.vector.tensor_copy(t1[:], t1p[:])
                # Gsl[:,0,:] = a0*r0 + (1-a0)*t1 ; Gsl[:,1,:] = a1*r1 + (1-a1)*t0
                Gsl = pconst.tile([B, 2, 128], bf16)
                gtmp = psmall.tile([B, 128], bf16, tag="gtmp")
                nc.vector.tensor_scalar(gtmp[:], t1[:], axf[:, 11:12], None,
                                        Alu.mult)
                nc.vector.scalar_tensor_tensor(Gsl[:, 0, :], r0[:], axf[:, 10:11],
                                               gtmp[:], Alu.mult, Alu.add)
                gtmp2 = psmall.tile([B, 128], bf16, tag="gtmp")
                nc.vector.tensor_scalar(gtmp2[:], t0[:], axf[:, 13:14], None,
                                        Alu.mult)
                nc.vector.scalar_tensor_tensor(Gsl[:, 1, :], r1[:], axf[:, 12:13],
                                               gtmp2[:], Alu.mult, Alu.add)

                # diag6 + per-slot D2 / DH via shipped selection masks
                diag6 = pconst.tile([B, 6], bf16)
                nc.vector.tensor_copy(diag6[:, 0:4], c1post[:, 193:197])
                nc.vector.tensor_copy(diag6[:, 4:6], c2post[:, 64:66])
                D2 = pconst.tile([B, 2], f32)
                DH = pconst.tile([B, 2], f32)
                for s in range(2):
                    selr = axb[:, 4 + 6 * s:10 + 6 * s]
                    selc = axb[:, 16 + 6 * s:22 + 6 * s]
                    dt_ = psmall.tile([B, 6], bf16, tag="dt")
                    nc.vector.tensor_mul(dt_[:], diag6[:], selr)
                    nc.vector.reduce_sum(out=D2[:, s:s + 1], in_=dt_[:], axis=X)
                    dt2 = psmall.tile([B, 6], bf16, tag="dt")
                    nc.vector.tensor_mul(dt2[:], diag6[:], selc)
                    nc.vector.reduce_sum(out=DH[:, s:s + 1], in_=dt2[:], axis=X)

                F = pconst.tile([B, 2], f32)
                nc.vector.memset(F[:], 0.0)
                sv = pconst.tile([B, 2], f32)
                mv = pconst.tile([B, 2], f32)
                scrx = pconst.tile([B, 2, 128], bf16)
                escr = pconst.tile([B, 128], bf16)

                # ln on DVE: exponent/mantissa split + deg-2 poly, max err 6e-3
                # (plenty for eps*ln; keeps the scalar engine Exp-only ->
                # no act-table reloads)
                LN2 = 0.6931471805599453
                LC2, LC1, LC0 = (-0.23351351824407424, 1.3827825718019444,
                                 -1.1430148212645563)

                def dve_ln(dst, src, n):
                    svi = src.bitcast(i32)
                    sh = psmall.tile([B, n], i32, tag="lsh")
                    nc.vector.tensor_scalar(sh[:], svi, 23, None,
                                            Alu.logical_shift_right)
                    ef = psmall.tile([B, n], f32, tag="lef")
                    nc.vector.tensor_copy(ef[:], sh[:])
                    mi = psmall.tile([B, n], i32, tag="lmi")
                    nc.vector.tensor_scalar(mi[:], svi, 0x007FFFFF, 0x3F800000,
                                            Alu.bitwise_and, Alu.bitwise_or)
                    t1 = psmall.tile([B, n], f32, tag="lt1")
                    nc.vector.tensor_scalar(t1[:], mi[:].bitcast(f32), LC2, LC1,
                                            Alu.mult, Alu.add)
                    t2 = psmall.tile([B, n], f32, tag="lt2")
                    nc.vector.tensor_tensor(t2[:], t1[:], mi[:].bitcast(f32),
                                            Alu.mult)
                    e2f = psmall.tile([B, n], f32, tag="le2")
                    nc.vector.tensor_scalar(e2f[:], ef[:], LN2,
                                            -127.0 * LN2 + LC0,
                                            Alu.mult, Alu.add)
                    nc.vector.tensor_tensor(dst, e2f[:], t2[:], Alu.add)

                for eps in _eps_schedule():
                    damp = 1.0 / (1.0 + eps / RHO)
                    c = GSCALE / eps
                    fsum = psmall.tile([B, 2], f32, tag="fsum")
                    nc.vector.tensor_add(fsum[:], F[:], DH[:])
                    ftp = fpsum.tile([2, 128], f32, tag="ft")
                    nc.tensor.transpose(ftp[:], fsum[:], identf[:])
                    HT = psmall.tile([2, 128], bf16, tag="ht")
                    nc.vector.tensor_scalar(HT[:], ftp[:], 1.0 / GSCALE,
                                            blog * eps / GSCALE,
                                            Alu.mult, Alu.add)
                    rhm = psmall.tile([2, 2, 128], bf16, tag="rhm")
                    nc.vector.tensor_tensor(
                        rhm[:], HT[:].unsqueeze(1).broadcast_to((2, 2, 128)),
                        mskt[:].rearrange("k (a j) -> k a j", j=128), Alu.mult)
                    hbt = hpsum.tile([128, 256], f32, tag="hb")
                    nc.tensor.matmul(hbt[:], ones2b[:],
                                     rhm[:].rearrange("k a j -> k (a j)"),
                                     start=True, stop=False)
                    nc.tensor.matmul(hbt[:], identb[:],
                                     Gsl[:].rearrange("b a j -> b (a j)"),
                                     start=False, stop=True)
                    hb3 = hbt[:].rearrange("b (s j) -> b s j", j=128)
                    nc.vector.reduce_max(out=mv[:], in_=hb3, axis=X)
                    nc.vector.tensor_tensor(
                        scrx[:], hb3,
                        mv[:].unsqueeze(2).broadcast_to((B, 2, 128)),
                        Alu.subtract)
                    for s in range(2):
                        nc.scalar.activation(escr[:], scrx[:, s, :], Act.Exp,
                                             scale=float(c),
                                             accum_out=sv[:, s:s + 1])
                    lg = psmall.tile([B, 2], f32, tag="lg")
                    dve_ln(lg[:], sv[:], 2)
                    # dmu = D2 - eps*lg - GSCALE*mv
                    dm1 = psmall.tile([B, 2], f32, tag="dm1")
                    nc.vector.scalar_tensor_tensor(dm1[:], lg[:], float(-eps),
                                                   D2[:], Alu.mult, Alu.add)
                    dmu = psmall.tile([B, 2], f32, tag="dmu")
                    nc.vector.scalar_tensor_tensor(dmu[:], mv[:], float(-GSCALE),
                                                   dm1[:], Alu.mult, Alu.add)
                    dr = psmall.tile([B, 2], f32, tag="dr")
                    nc.vector.tensor_copy(dr[:, 0:1], dmu[:, 1:2])
                    nc.vector.tensor_copy(dr[:, 1:2], dmu[:, 0:1])
                    # cmix = damp * ((1-pf)*dmu + pf*rev(dmu))
                    c1t = psmall.tile([B, 2], f32, tag="c1t")
                    nc.vector.tensor_scalar(c1t[:], dmu[:], axf[:, 1:2],
                                            float(damp), Alu.mult, Alu.mult)
                    c2t = psmall.tile([B, 2], f32, tag="c2t")
                    nc.vector.tensor_scalar(c2t[:], dr[:], axf[:, 0:1],
                                            float(damp), Alu.mult, Alu.mult)
                    cmix = psmall.tile([B, 2], f32, tag="cmix")
                    nc.vector.tensor_add(cmix[:], c1t[:], c2t[:])
                    # F = wF*F + vF*cmix
                    m1 = psmall.tile([B, 2], f32, tag="m1")
                    nc.vector.tensor_mul(m1[:], F[:], axf[:, 2:4])
                    m2 = psmall.tile([B, 2], f32, tag="m2")
                    nc.vector.tensor_mul(m2[:], cmix[:], axf[:, 4:6])
                    nc.vector.tensor_add(F[:], m1[:], m2[:])

                # ---- loss_kd partial ----
                E2 = psmall.tile([B, 2], f32, tag="e2")
                nc.scalar.activation(E2[:], F[:], Act.Exp, scale=float(-1.0 / RHO))
                km = psmall.tile([B, 2], f32, tag="km")
                nc.vector.tensor_mul(km[:], E2[:], axf[:, 6:8])
                kdp = psmall.tile([B, 1], f32, tag="kdp")
                nc.vector.reduce_sum(out=kdp[:], in_=km[:], axis=X)

                # ---- CE (replicated; gated by aux csup/cemb) ----
                pcall = pconst.tile([B, 192], f32)
                nc.vector.tensor_copy(pcall[:, 0:128], c1post[:, 0:128])
                nc.vector.tensor_copy(pcall[:, 128:192], c2post[:, 0:64])
                af = pconst.tile([B, 64], f32)
                nc.vector.tensor_copy(af[:], c1post[:, 128:192])
                embcol = pconst.tile([B, 1], f32)
                nc.vector.tensor_copy(embcol[:], c1post[:, 192:193])

                idxf = pconst.tile([B, 64], f32)
                nc.scalar.dma_start(out=idxf[:], in_=idx_dram[:, :])
                pos = psmall.tile([B, 64], f32, tag="pos")
                nc.vector.tensor_scalar(pos[:], pcall[:, 0:64], 0.0, None,
                                        Alu.is_gt)
                ip1 = psmall.tile([B, 64], f32, tag="ip1")
                nc.vector.scalar_tensor_tensor(ip1[:], idxf[:], 1.0, pos[:],
                                               Alu.add, Alu.mult)
                Lp = psmall.tile([B, 1], f32, tag="Lp")
                nc.vector.reduce_max(out=Lp[:], in_=ip1[:], axis=X)
                eq0 = psmall.tile([B, 1], f32, tag="eq0")
                nc.vector.tensor_scalar(eq0[:], Lp[:], 0.0, None, Alu.is_equal)
                Lv = psmall.tile([B, 1], f32, tag="Lv")
                nc.vector.scalar_tensor_tensor(Lv[:], eq0[:], float(S), Lp[:],
                                               Alu.mult, Alu.add)
                dl = psmall.tile([B, 64], f32, tag="dl")
                nc.vector.tensor_scalar(dl[:], idxf[:], Lv[:, 0:1], None,
                                        Alu.subtract)
                mask = psmall.tile([B, 64], f32, tag="mask")
                nc.vector.tensor_scalar(mask[:], dl[:], 0.0, None, Alu.is_lt)
                negf = psmall.tile([B, 64], f32, tag="negf")
                nc.vector.tensor_scalar(negf[:], mask[:], 1.0, 1e9,
                                        Alu.subtract, Alu.mult)
                # a = floor((asum+1)/2) via magic round (values < 2^22)
                MAGIC = 12582912.0
                tv = psmall.tile([B, 64], f32, tag="tv")
                nc.vector.tensor_scalar(tv[:], af[:], 0.5, 1024.25,
                                        Alu.mult, Alu.add)
                tm = psmall.tile([B, 64], f32, tag="tm")
                nc.vector.tensor_scalar(tm[:], tv[:], MAGIC, MAGIC,
                                        Alu.add, Alu.subtract)
                av = psmall.tile([B, 64], f32, tag="av")
                nc.vector.tensor_scalar(av[:], tm[:], 1024.0, None, Alu.subtract)
                amask = psmall.tile([B, 64], f32, tag="amask")
                nc.vector.tensor_tensor(amask[:], av[:], mask[:], Alu.mult)
                pc3 = pcall[:].rearrange("b (s q) -> b s q", q=64)
                mce = pbig.tile([B, 3, 64], f32, tag="mce")
                mask3 = mask[:].unsqueeze(1).broadcast_to((B, 3, 64))
                negf3 = negf[:].unsqueeze(1).broadcast_to((B, 3, 64))
                amask3 = amask[:].unsqueeze(1).broadcast_to((B, 3, 64))
                t2_ = pbig.tile([B, 3, 64], f32, tag="tt")
                nc.vector.scalar_tensor_tensor(t2_[:], pc3, 2.0, mask3, Alu.mult,
                                               Alu.mult)
                nc.vector.tensor_tensor(mce[:], t2_[:], negf3, Alu.add)
                mx3 = psmall.tile([B, 3], f32, tag="mx3")
                nc.vector.reduce_max(out=mx3[:], in_=mce[:], axis=X)
                mb3 = mx3[:].unsqueeze(2).broadcast_to((B, 3, 64))
                dd = pbig.tile([B, 3, 64], f32, tag="dd")
                nc.vector.tensor_tensor(dd[:], mce[:], mb3, Alu.subtract)
                ee = pbig.tile([B, 3, 64], f32, tag="ee")
                nc.scalar.activation(ee[:], dd[:], Act.Exp)
                ss3 = psmall.tile([B, 3], f32, tag="ss3")
                nc.vector.reduce_sum(out=ss3[:], in_=ee[:], axis=X)
                lg3 = psmall.tile([B, 3], f32, tag="lg3")
                dve_ln(lg3[:], ss3[:], 3)
                lse3 = psmall.tile([B, 3], f32, tag="lse3")
                nc.vector.tensor_add(lse3[:], mx3[:], lg3[:])
                lb3 = lse3[:].unsqueeze(2).broadcast_to((B, 3, 64))
                d1 = pbig.tile([B, 3, 64], f32, tag="dd")
                nc.vector.tensor_tensor(d1[:], mce[:], lb3, Alu.subtract)
                d2_ = pbig.tile([B, 3, 64], f32, tag="tt")
                nc.vector.tensor_tensor(d2_[:], d1[:], amask3, Alu.mult)
                rowsum = psmall.tile([B, 1], f32, tag="rs")
                nc.vector.reduce_sum(out=rowsum[:],
                                     in_=d2_[:].rearrange("b s q -> b (s q)"),
                                     axis=X)

                # ---- final combine: csup*CE + cemb*embed + kd_partial ----
                tot_ps = spsum.tile([1, 1], f32, tag="tot")
                nc.tensor.matmul(tot_ps[:], rowsum[:], axf[:, 8:9], start=True,
                                 stop=False)
                nc.tensor.matmul(tot_ps[:], embcol[:], axf[:, 9:10], start=False,
                                 stop=False)
                nc.tensor.matmul(tot_ps[:], kdp[:], ones_col[:], start=False,
                                 stop=True)
                outt = psmall.tile([1, 1], f32, tag="outt")
                nc.vector.tensor_copy(outt[:], tot_ps[:])
                nc.sync.dma_start(out=out_ext[:, :], in_=outt[:])

    nc.compile()
    return nc


_NC = None
LAST_RESULTS = None


def _core_aux(c):
    sl = SLOTS[c]
    i0 = C1MAT[sl['i0']] if sl['i0'] is not None else 0
    i1 = C2MAT[sl['i1']] if sl['i1'] is not None else 0
    pf = float(sl['pf'])
    wf = [0.0, 0.0] if sl['pf'] else [0.5, 0.5]
    vf = [1.0, 1.0] if sl['pf'] else [0.5, 0.5]
    cgate = 1.0 if c == 0 else 0.0
    auxf = np.zeros((B, 14), np.float32)
    auxf[:, 0] = pf
    auxf[:, 1] = 1.0 - pf
    auxf[:, 2:4] = wf
    auxf[:, 4:6] = vf
    auxf[:, 6:8] = sl['kc']
    auxf[:, 8] = -LOSS_WEIGHT * SUP_W * cgate
    auxf[:, 9] = LOSS_WEIGHT * EMBED_W * 0.5 * cgate
    auxf[:, 10] = float(sl['a0'])
    auxf[:, 11] = 1.0 - float(sl['a0'])
    auxf[:, 12] = float(sl['a1'])
    auxf[:, 13] = 1.0 - float(sl['a1'])
    auxb = np.zeros((B, 28), np.float32)
    auxb[:, 0] = float(sl['a0'])
    auxb[:, 1] = 1.0 - float(sl['a0'])
    auxb[:, 2] = float(sl['a1'])
    auxb[:, 3] = 1.0 - float(sl['a1'])
    for s in range(2):
        auxb[:, 4 + 6 * s + DIDX[sl['rs'][s]]] = 2.0
        auxb[:, 16 + 6 * s + DIDX[sl['cs'][s]]] = -2.0
    auxi = np.zeros((B, 2), np.int32)
    auxi[:, 0] = 128 * i0 + np.arange(B)
    auxi[:, 1] = 128 * i1 + np.arange(B)
    return auxf, auxb, auxi


def _shard_inputs(logit_c, logit_t, logit_ensemble, logit_teacher_c,
                  logit_teacher_t, logit_teacher_ensemble, out_h_student,
                  out_h_teacher, out_d_student, out_d_teacher, batch):
    import ml_dtypes
    bf = ml_dtypes.bfloat16
    students = [logit_c, logit_t, logit_ensemble]
    teachers = [logit_teacher_c, logit_teacher_t, logit_teacher_ensemble]
    embeds = dict(ehs=out_h_student, eht=out_h_teacher,
                  eds=out_d_student, edt=out_d_teacher)
    # q-major [B, QS, T] bf16 per core (XBAR transpose source) + t-major
    # [B, S, QS] copies for the contiguous CE/delta path
    sbf = [np.asarray(a, np.float32).astype(bf) for a in students]
    sb = [np.ascontiguousarray(np.transpose(a, (0, 2, 1))) for a in sbf]
    tb = [np.ascontiguousarray(np.transpose(
        np.asarray(a, np.float32).astype(bf), (0, 2, 1))) for a in teachers]
    bct = np.asarray(batch[:, 1:1 + S, :Q], np.float32).astype(bf)
    bnt = np.asarray(batch[:, 1:1 + S, Q:], np.float32).astype(bf)
    in_maps = []
    for c in range(NCORES):
        q0 = QS * c
        m = {}
        for nm, arr in zip(("xc", "xt", "xe"), sb):
            m[nm] = np.ascontiguousarray(arr[:, q0:q0 + QS, :].transpose(1, 2, 0))
        for nm, arr in zip(("yc", "yt", "ye"), tb):
            m[nm] = np.ascontiguousarray(arr[:, q0:q0 + QS, :].transpose(1, 2, 0))
        for nm, arr in zip(("xct", "xtt", "xet"), sbf):
            m[nm] = np.ascontiguousarray(arr[:, 0:S, q0:q0 + QS])
        m["dbc"] = np.ascontiguousarray(bct[:, :, q0:q0 + QS])
        m["dbn"] = np.ascontiguousarray(bnt[:, :, q0:q0 + QS])
        t0, w = EOFF[c], ESPLIT[c]
        for nm, arr in embeds.items():
            sl = np.zeros((B, EPAD, H), bf)
            sl[:, :w, :] = np.asarray(arr[:, t0:t0 + w, :], np.float32).astype(bf)
            m[nm] = sl
        axf, axb, axi = _core_aux(c)
        m["auxf"] = axf
        m["auxb"] = axb.astype(bf)
        m["auxi"] = axi
        in_maps.append(m)
    return in_maps


def kernel(**inputs):
    global _NC, LAST_RESULTS
    from concourse.bass_utils import run_bass_kernel_spmd
    if _NC is None:
        _NC = build_bass()
    in_maps = _shard_inputs(**inputs)
    trace = bool(int(os.environ.get("KERNEL_TRACE", "0")))
    res = run_bass_kernel_spmd(_NC, in_maps, list(range(NCORES)), trace=trace)
    LAST_RESULTS = res
    total = sum(float(np.asarray(r["out"]).reshape(-1)[0]) for r in res.results)
    return np.asarray([total], dtype=np.float32)
